# revision 1
# baseline (speedup 1.0000x reference)
"""GCN block (GraphConv + BatchNorm1d + ReLU) on 8 Trainium2 NeuronCores.

Strategy (per sharding hint): partition nodes (and incident edges) across the
8 cores; replicate W/b/gamma/beta; all-reduce BN batch statistics.

Per core k (owns dst nodes [k*NPC, (k+1)*NPC)):
  1. h_k = (x_k @ W) * rsqrt(clip(deg_out_k,1))           (PE matmul, fp32)
  2. AllGather h (bf16) -> full h table in every core's HBM
  3. For each 128-node dst group, gather h[src] rows of the group's edges
     (dma_gather, bf16, batched) and segment-sum them with one-hot matmuls
     M^T @ G accumulated in PSUM (avoids dma_scatter_add, which loses
     updates on duplicate indices - verified on HW).
  4. relu(agg * rsqrt(clip(deg_in,1)) + b); local BN sums; AllReduce sums;
     y = (h - mu) * rsqrt(var+eps) * gamma + beta.

Host-side work is limited to integer index bookkeeping (bucketing edges by
(core, src-bank, dst-group), degree counting) and layout transforms (x^T,
int16 gather indices). All floating-point math runs on device.

Edges are bucketed by src bank (4 banks of N/4 rows) because dma_gather
indices are int16 (< 32768). Bucket sizes are padded to a structure shared
by all 8 cores so a single SPMD NEFF serves every core; pad slots gather row
0 of the bank and carry a dst offset of 255 -> their one-hot column is all
zeros, so they contribute exactly 0.
"""
import math
import os
import sys

sys.path.insert(0, "/opt/trn_rl_repo")

import numpy as np

import concourse.bacc as bacc
import concourse.bass as bass
import concourse.mybir as mybir
import concourse.tile as tile
from concourse import bass_utils

F32 = mybir.dt.float32
BF16 = mybir.dt.bfloat16
I16 = mybir.dt.int16

CFG = dict(
    N=100000,
    E=1600000,
    IN=256,
    OUT=128,
    NCORES=8,
    GRP=128,          # dst nodes per segment group (= psum partition dim)
    NBANKS=4,         # src banks (bank rows must stay < 32768 for int16 idx)
    BATCH_BLOCKS=40,  # gather batch size in 128-edge blocks
    EPS=1e-5,
    TRACE=False,
)

LAST_RESULTS = None  # set by kernel() for test harness introspection
LAST_NC = None
LAST_RUN_S = None


def _ceil_div(a, b):
    return (a + b - 1) // b


def _wrap16(idx, ncols):
    """int16 idx list -> [128, ncols] tile: idx i at [i%16, i//16], replicated
    8x across the 16-partition groups (one copy per GpSimd Q7 core)."""
    n = idx.shape[0]
    assert n == ncols * 16
    w = np.ascontiguousarray(idx.reshape(ncols, 16).T)
    return np.tile(w, (8, 1))


def _preprocess(cfg, src, dst):
    """Bucket edges by (owner core, src bank, dst group); build per-core
    gather-index / dst-offset arrays and the shared block structure."""
    N, E = cfg["N"], cfg["E"]
    C, NBANKS, GRP = cfg["NCORES"], cfg["NBANKS"], cfg["GRP"]
    NPC = N // C
    NG = _ceil_div(NPC, GRP)
    assert NPC % NBANKS == 0
    QROWS = NPC // NBANKS          # rows per quarter of a core's shard
    BANKROWS = QROWS * C           # rows per bank table (one AllGather output)
    assert BANKROWS < 32768

    src = src.astype(np.int64)
    dst = dst.astype(np.int64)
    deg_out = np.bincount(src, minlength=N).astype(np.float32)
    deg_in = np.bincount(dst, minlength=N).astype(np.float32)

    owner = dst // NPC
    bank = (src % NPC) // QROWS    # quarter index within the source's shard
    grp = (dst % NPC) // GRP
    key = (owner * NBANKS + bank) * NG + grp
    order = np.argsort(key, kind="stable")
    s_src = src[order]
    s_dst = dst[order]
    s_key = key[order]

    counts = np.bincount(key, minlength=C * NBANKS * NG).reshape(C, NBANKS, NG)
    P = counts.max(axis=0)  # [NBANKS, NG]
    P = ((P + 127) // 128) * 128
    P[0] = np.maximum(P[0], 128)  # bank-0 run always exists (initializes agg)

    nidx_tot = int(P.sum())
    nb_tot = nidx_tot // 128
    # stream order: group-chunks outer, banks inner -> the ReLU/BN stage of a
    # chunk's groups can overlap later chunks' gathers
    GC = cfg.get("GCHUNK", 13)
    chunks = [list(range(c, min(c + GC, NG))) for c in range(0, NG, GC)]
    run_seq = [(b, g) for ch in chunks for b in range(NBANKS) for g in ch]
    run_off = np.zeros((NBANKS, NG), np.int64)
    pos = 0
    for b, g in run_seq:
        run_off[b, g] = pos
        pos += P[b, g]

    # boundaries of each (k, b, g) bucket in the sorted edge stream
    bkeys = (np.arange(C)[:, None, None] * NBANKS + np.arange(NBANKS)[None, :, None]) * NG + np.arange(NG)[None, None, :]
    starts = np.searchsorted(s_key, bkeys.ravel()).reshape(C, NBANKS, NG)
    ends = np.searchsorted(s_key, bkeys.ravel(), side="right").reshape(C, NBANKS, NG)

    gidx_cores = []
    dstoff_cores = []
    for k in range(C):
        gidx = np.zeros(nidx_tot, np.int16)
        doff = np.full(nidx_tot, 255.0, np.float32)
        for b in range(NBANKS):
            for g in range(NG):
                s, e = starts[k, b, g], ends[k, b, g]
                cnt = e - s
                if cnt == 0:
                    continue
                p0 = run_off[b, g]
                gidx[p0 : p0 + cnt] = (
                    (s_src[s:e] // NPC) * QROWS + (s_src[s:e] % NPC) % QROWS
                ).astype(np.int16)
                doff[p0 : p0 + cnt] = ((s_dst[s:e] % NPC) - g * GRP).astype(np.float32)
        gidx_cores.append(_wrap16(gidx, nidx_tot // 16))
        # dstoff tile [128, nb_tot]: col t = offsets of block t's 128 edges
        dstoff_cores.append(np.ascontiguousarray(doff.reshape(nb_tot, 128).T))

    # shared static block structure: per block t -> (bank, group, start, stop)
    blocks = []
    for b, g in run_seq:
        nb = P[b, g] // 128
        for j in range(nb):
            blocks.append((b, g, j == 0, j == nb - 1))

    # gather batches: consecutive blocks within one bank, <= BATCH_BLOCKS
    batches = []  # (bank, first_block, n_blocks)
    t = 0
    while t < len(blocks):
        b = blocks[t][0]
        n = 1
        while (
            t + n < len(blocks)
            and blocks[t + n][0] == b
            and n < cfg["BATCH_BLOCKS"]
        ):
            n += 1
        batches.append((b, t, n))
        t += n

    meta = dict(
        NPC=NPC,
        NG=NG,
        BANKROWS=BANKROWS,
        QROWS=QROWS,
        nidx_tot=nidx_tot,
        nb_tot=nb_tot,
        blocks=blocks,
        batches=batches,
        deg_out=deg_out,
        deg_in=deg_in,
    )
    return meta, gidx_cores, dstoff_cores


def _tile_major(vec, NG, GRP, pad_val):
    """[NPC] -> [GRP, NG]: entry (p, m) = vec[m*GRP + p], padded."""
    out = np.full((NG * GRP,), pad_val, vec.dtype)
    out[: vec.shape[0]] = vec
    return np.ascontiguousarray(out.reshape(NG, GRP).T)


def _build_nc(cfg, meta):
    N, IN, OUT, C = cfg["N"], cfg["IN"], cfg["OUT"], cfg["NCORES"]
    GRP, NBANKS = cfg["GRP"], cfg["NBANKS"]
    NPC, NG, BANKROWS = meta["NPC"], meta["NG"], meta["BANKROWS"]
    QROWS = meta["QROWS"]
    nidx_tot, nb_tot = meta["nidx_tot"], meta["nb_tot"]
    blocks, batches = meta["blocks"], meta["batches"]
    XK = _ceil_div(IN, 128)
    assert OUT == 128 and GRP == 128
    last_w = NPC - (NG - 1) * GRP  # valid rows in the last group

    nc = bacc.Bacc(
        "TRN2", target_bir_lowering=False, debug=False, num_devices=C
    )

    # ---- external inputs ----
    xt = [
        nc.dram_tensor(f"xt{j}", [128, NPC], BF16, kind="ExternalInput")
        for j in range(XK)
    ]
    wt = [
        nc.dram_tensor(f"wt{j}", [128, OUT], BF16, kind="ExternalInput")
        for j in range(XK)
    ]
    gidx_d = nc.dram_tensor("gidx", [128, nidx_tot // 16], I16, kind="ExternalInput")
    doff_d = nc.dram_tensor("doff", [128, nb_tot], F32, kind="ExternalInput")
    dego_d = nc.dram_tensor("dego", [128, NG], F32, kind="ExternalInput")
    degi_d = nc.dram_tensor("degi", [128, NG], F32, kind="ExternalInput")
    bt_d = nc.dram_tensor("bt", [128, OUT], F32, kind="ExternalInput")
    iota_d = nc.dram_tensor("iota", [128, GRP], BF16, kind="ExternalInput")
    gm_d = nc.dram_tensor("gm", [1, OUT], F32, kind="ExternalInput")
    bb_d = nc.dram_tensor("bb", [1, OUT], F32, kind="ExternalInput")
    onesc_d = nc.dram_tensor("onesc", [128, 1], F32, kind="ExternalInput")
    onest_d = nc.dram_tensor("onest", [128, 1], F32, kind="ExternalInput")
    onesr_d = nc.dram_tensor("onesr", [1, 128], F32, kind="ExternalInput")

    ypad_d = nc.dram_tensor("ypad", [NG * GRP, OUT], F32, kind="ExternalOutput")

    with tile.TileContext(nc) as tc:
        with (
            tc.tile_pool(name="const", bufs=1) as cpool,
            tc.tile_pool(name="dram", bufs=1, space="DRAM") as dpool,
            tc.tile_pool(name="agg", bufs=1) as apool,
            tc.tile_pool(name="gath", bufs=3) as gpool,
            tc.tile_pool(name="mpool", bufs=6) as mpool,
            tc.tile_pool(name="etmp", bufs=4) as epool,
            tc.tile_pool(name="gtmp", bufs=4) as gpool2,
            tc.tile_pool(name="psum", bufs=3, space="PSUM") as ppool,
            tc.tile_pool(name="pstat", bufs=1, space="PSUM") as pspool,
        ):
            # ---- constants / small tiles ----
            iota_t = cpool.tile([128, GRP], BF16)
            bt_t = cpool.tile([128, OUT], F32)
            dego_t = cpool.tile([128, NG], F32)
            degi_t = cpool.tile([128, NG], F32)
            nsrc_t = cpool.tile([128, NG], F32)
            ndst_t = cpool.tile([128, NG], F32)
            gm_t = cpool.tile([1, OUT], F32)
            bb_t = cpool.tile([1, OUT], F32)
            onesc_t = cpool.tile([128, 1], F32)
            onest_t = cpool.tile([128, 1], F32)
            onesr_t = cpool.tile([1, 128], F32)
            gidx_t = cpool.tile([128, nidx_tot // 16], I16)
            doff_t = cpool.tile([128, nb_tot], F32)

            nc.sync.dma_start(iota_t[:], iota_d[:])
            nc.sync.dma_start(bt_t[:], bt_d[:])
            nc.sync.dma_start(dego_t[:], dego_d[:])
            nc.sync.dma_start(degi_t[:], degi_d[:])
            nc.sync.dma_start(gm_t[:], gm_d[:])
            nc.sync.dma_start(bb_t[:], bb_d[:])
            nc.sync.dma_start(onesc_t[:], onesc_d[:])
            nc.sync.dma_start(onest_t[:], onest_d[:])
            nc.sync.dma_start(onesr_t[:], onesr_d[:])
            nc.sync.dma_start(gidx_t[:], gidx_d[:])
            nc.sync.dma_start(doff_t[:], doff_d[:])

            # norms: rsqrt(max(deg, 1))
            for deg_t, norm_t in ((dego_t, nsrc_t), (degi_t, ndst_t)):
                nc.vector.tensor_scalar(
                    norm_t[:], deg_t[:], 1.0, None, op0=mybir.AluOpType.max
                )
                nc.vector.reciprocal(norm_t[:], norm_t[:])
                nc.scalar.activation(
                    norm_t[:], norm_t[:], mybir.ActivationFunctionType.Sqrt
                )

            # internal DRAM for collectives (quartered for B/C/D pipelining)
            h_my_qs = [
                dpool.tile([QROWS, OUT], BF16, name=f"h_my_{q}")
                for q in range(NBANKS)
            ]
            _aspace = "Local" if cfg.get("NOCC") else "Shared"
            h_all_qs = [
                dpool.tile([BANKROWS, OUT], BF16, addr_space=_aspace, name=f"h_all_{q}")
                for q in range(NBANKS)
            ]
            stats_in = dpool.tile([1, 2 * OUT], F32)
            stats_out = dpool.tile([1, 2 * OUT], F32, addr_space=_aspace)

            agg_t = apool.tile([128, NG, OUT], F32)

            # ---- stage B: h = (x @ W) * norm_src, cast bf16, store to HBM
            with tc.tile_pool(name="xw", bufs=1) as xwp, tc.tile_pool(
                name="hbf", bufs=4
            ) as hbp:
                xts = []
                wts = []
                for j in range(XK):
                    xtile = xwp.tile([128, NPC], BF16, name=f"xt_s{j}")
                    wtile = xwp.tile([128, OUT], BF16, name=f"wt_s{j}")
                    nc.sync.dma_start(xtile[:], xt[j][:])
                    nc.sync.dma_start(wtile[:], wt[j][:])
                    xts.append(xtile)
                    wts.append(wtile)
                for m in range(NG):
                    w = GRP if m < NG - 1 else last_w
                    ps = ppool.tile([128, OUT], F32, tag="hps")
                    for j in range(XK):
                        nc.tensor.matmul(
                            ps[:w, :],
                            xts[j][:, m * GRP : m * GRP + w],
                            wts[j][:, :],
                            start=(j == 0),
                            stop=(j == XK - 1),
                        )
                    hb = hbp.tile([128, OUT], BF16, tag="hb")
                    nc.scalar.activation(
                        hb[:w, :],
                        ps[:w, :],
                        mybir.ActivationFunctionType.Copy,
                        scale=nsrc_t[:w, m : m + 1],
                    )
                    r0 = m * GRP
                    r1 = r0 + w
                    q0 = r0 // QROWS
                    q1 = (r1 - 1) // QROWS
                    for q in range(q0, q1 + 1):
                        a = max(r0, q * QROWS)
                        z = min(r1, (q + 1) * QROWS)
                        nc.sync.dma_start(
                            h_my_qs[q][a - q * QROWS : z - q * QROWS, :],
                            hb[a - r0 : z - r0, :],
                        )

            # ---- stage C: quartered AllGather (pipelines with B and D) ----
            for q in range(NBANKS):
                if cfg.get("NOCC"):
                    rep = (
                        h_my_qs[q][:]
                        .rearrange("(o r) f -> o r f", o=1)
                        .to_broadcast((C, QROWS, OUT))
                    )
                    nc.sync.dma_start(
                        h_all_qs[q][:].rearrange("(o r) f -> o r f", o=C), rep
                    )
                else:
                    nc.gpsimd.collective_compute(
                        "AllGather",
                        mybir.AluOpType.bypass,
                        replica_groups=[list(range(C))],
                        ins=[h_my_qs[q][:]],
                        outs=[h_all_qs[q][:]],
                    )

            # ---- stage D: gather + one-hot matmul segmented sum ----
            stages = cfg.get("STAGES", "BCDEFG")
            if "D" not in stages or cfg.get("DSUB", 3) < 3:
                nc.gpsimd.memset(agg_t[:], 0.0)
            if "D" in stages:
              if True:
                  ps_run = None
                  bmax = max(nb for _, _, nb in batches)
                  for bank, t0, nblk in batches:
                      Gt = gpool.tile([128, bmax, OUT], BF16, tag="G")
                      nc.gpsimd.dma_gather(
                          Gt[:, :nblk, :],
                          h_all_qs[bank][:],
                          gidx_t[:, t0 * 8 : (t0 + nblk) * 8],
                          nblk * 128,
                          nblk * 128,
                          OUT,
                          single_packet=False,
                      )
                      for j in range(nblk):
                          if cfg.get("DSUB", 3) < 2:
                              continue
                          t = t0 + j
                          b, g, is_start, is_stop = blocks[t]
                          Mt = mpool.tile([128, GRP], BF16, tag="M")
                          nc.vector.tensor_scalar(
                              Mt[:],
                              iota_t[:],
                              doff_t[:, t : t + 1],
                              None,
                              op0=mybir.AluOpType.is_equal,
                          )
                          if cfg.get("DSUB", 3) < 3:
                              continue
                          if is_start:
                              ps_run = ppool.tile([128, OUT], F32, tag="aggps")
                          nc.tensor.matmul(
                              ps_run[:],
                              Mt[:],
                              Gt[:, j, :],
                              start=is_start,
                              stop=is_stop,
                          )
                          if is_stop:
                              if b == 0:
                                  nc.scalar.activation(
                                      agg_t[:, g, :],
                                      ps_run[:],
                                      mybir.ActivationFunctionType.Copy,
                                  )
                              else:
                                  nc.vector.tensor_tensor(
                                      agg_t[:, g, :],
                                      agg_t[:, g, :],
                                      ps_run[:],
                                      op=mybir.AluOpType.add,
                                  )

            # ---- stage E: relu(agg*norm_dst + b); BN partial sums ----
            ps_sum = pspool.tile([1, OUT], F32, name="ps_sum")
            ps_sq = pspool.tile([1, OUT], F32, name="ps_sq")
            if "E" in stages:
              if True:
                  for g in range(NG):
                      tmp = epool.tile([128, OUT], F32, tag="etmp")
                      nc.vector.scalar_tensor_tensor(
                          tmp[:],
                          agg_t[:, g, :],
                          ndst_t[:, g : g + 1],
                          bt_t[:],
                          op0=mybir.AluOpType.mult,
                          op1=mybir.AluOpType.add,
                      )
                      nc.scalar.activation(
                          agg_t[:, g, :], tmp[:], mybir.ActivationFunctionType.Relu
                      )
                      ones = onesc_t if g < NG - 1 else onest_t
                      nc.tensor.matmul(
                          ps_sum[:],
                          ones[:],
                          agg_t[:, g, :],
                          start=(g == 0),
                          stop=(g == NG - 1),
                      )
                      sq = epool.tile([128, OUT], F32, tag="esq")
                      nc.scalar.activation(
                          sq[:], agg_t[:, g, :], mybir.ActivationFunctionType.Square
                      )
                      nc.tensor.matmul(
                          ps_sq[:],
                          ones[:],
                          sq[:],
                          start=(g == 0),
                          stop=(g == NG - 1),
                      )

            # ---- stage F: AllReduce BN stats; build affine S/T tiles ----
            S_t = cpool.tile([128, OUT], F32)
            T_t = cpool.tile([128, OUT], F32)
            if "F" not in stages:
                nc.gpsimd.memset(S_t[:], 1.0)
                nc.gpsimd.memset(T_t[:], 0.0)
            if "F" in stages:
              st_sb = cpool.tile([1, 2 * OUT], F32)
              nc.scalar.activation(
                  st_sb[:, 0:OUT], ps_sum[:], mybir.ActivationFunctionType.Copy
              )
              nc.scalar.activation(
                  st_sb[:, OUT : 2 * OUT], ps_sq[:], mybir.ActivationFunctionType.Copy
              )
              nc.sync.dma_start(stats_in[:], st_sb[:])
              if cfg.get("NOCC"):
                  nc.sync.dma_start(stats_out[:], stats_in[:])
              else:
                  nc.gpsimd.collective_compute(
                      "AllReduce",
                      mybir.AluOpType.add,
                      replica_groups=[list(range(C))],
                      ins=[stats_in[:]],
                      outs=[stats_out[:]],
                  )
              st_rb = cpool.tile([1, 2 * OUT], F32)
              nc.sync.dma_start(st_rb[:], stats_out[:])

              mu = cpool.tile([1, OUT], F32)
              ex2 = cpool.tile([1, OUT], F32)
              var = cpool.tile([1, OUT], F32)
              srow = cpool.tile([1, OUT], F32)
              trow = cpool.tile([1, OUT], F32)
              inv_n = 1.0 / float(N)
              nc.scalar.activation(
                  mu[:], st_rb[:, 0:OUT], mybir.ActivationFunctionType.Copy, scale=inv_n
              )
              nc.scalar.activation(
                  ex2[:], st_rb[:, OUT : 2 * OUT], mybir.ActivationFunctionType.Copy, scale=inv_n
              )
              nc.scalar.activation(
                  var[:], mu[:], mybir.ActivationFunctionType.Square
              )
              nc.vector.tensor_sub(var[:], ex2[:], var[:])
              # var <- rsqrt(var + eps) (ACT Rsqrt is banned for accuracy)
              nc.scalar.activation(
                  var[:],
                  var[:],
                  mybir.ActivationFunctionType.Copy,
                  bias=float(cfg["EPS"]),
              )
              nc.vector.reciprocal(var[:], var[:])
              nc.scalar.activation(
                  var[:], var[:], mybir.ActivationFunctionType.Sqrt
              )
              nc.vector.tensor_mul(srow[:], gm_t[:], var[:])
              nc.vector.tensor_mul(trow[:], mu[:], srow[:])
              nc.vector.tensor_sub(trow[:], bb_t[:], trow[:])

              ps_S = ppool.tile([128, OUT], F32, tag="aggps", name="ps_S")
              ps_T = ppool.tile([128, OUT], F32, tag="aggps", name="ps_T")
              nc.tensor.matmul(ps_S[:], onesr_t[:], srow[:], start=True, stop=True)
              nc.tensor.matmul(ps_T[:], onesr_t[:], trow[:], start=True, stop=True)
              nc.scalar.activation(
                  S_t[:], ps_S[:], mybir.ActivationFunctionType.Copy
              )
              nc.scalar.activation(
                  T_t[:], ps_T[:], mybir.ActivationFunctionType.Copy
              )

            # ---- stage G: y = hrelu * S + T, write out ----
            if True:
                for g in range(NG):
                    tmp = gpool2.tile([128, OUT], F32, tag="gtmp")
                    nc.vector.tensor_mul(tmp[:], agg_t[:, g, :], S_t[:])
                    nc.vector.tensor_add(agg_t[:, g, :], tmp[:], T_t[:])
                ypad_view = ypad_d[:].rearrange("(g p) f -> p g f", p=128)
                nc.sync.dma_start(ypad_view, agg_t[:, :, :])

    nc.compile()
    return nc


def kernel(x, src, dst, W, b, gamma, beta):
    global LAST_RESULTS
    cfg = CFG
    N, E, IN, OUT, C = cfg["N"], cfg["E"], cfg["IN"], cfg["OUT"], cfg["NCORES"]
    GRP = cfg["GRP"]
    assert x.shape == (N, IN) and W.shape == (IN, OUT)
    assert src.shape == (E,) and dst.shape == (E,)

    meta, gidx_cores, dstoff_cores = _preprocess(cfg, src, dst)
    NPC, NG = meta["NPC"], meta["NG"]
    XK = _ceil_div(IN, 128)
    last_w = NPC - (NG - 1) * GRP

    nc = _build_nc(cfg, meta)

    xT = np.ascontiguousarray(np.asarray(x, np.float32).T)  # [IN, N]
    Wn = np.asarray(W, np.float32)
    import ml_dtypes

    iota = np.tile(
        np.arange(GRP, dtype=np.float32)[None, :], (128, 1)
    ).astype(ml_dtypes.bfloat16)
    bt = np.tile(np.asarray(b, np.float32)[None, :], (128, 1))
    onesc = np.ones((128, 1), np.float32)
    onest = np.zeros((128, 1), np.float32)
    onest[:last_w] = 1.0
    onesr = np.ones((1, 128), np.float32)
    gm = np.asarray(gamma, np.float32)[None, :]
    bb = np.asarray(beta, np.float32)[None, :]

    in_maps = []
    for k in range(C):
        im = {
            "gidx": gidx_cores[k],
            "doff": dstoff_cores[k],
            "dego": _tile_major(
                meta["deg_out"][k * NPC : (k + 1) * NPC], NG, GRP, np.float32(1.0)
            ),
            "degi": _tile_major(
                meta["deg_in"][k * NPC : (k + 1) * NPC], NG, GRP, np.float32(1.0)
            ),
            "bt": bt,
            "iota": iota,
            "gm": gm,
            "bb": bb,
            "onesc": onesc,
            "onest": onest,
            "onesr": onesr,
        }
        for j in range(XK):
            im[f"xt{j}"] = np.ascontiguousarray(
                xT[j * 128 : (j + 1) * 128, k * NPC : (k + 1) * NPC]
            ).astype(ml_dtypes.bfloat16)
            im[f"wt{j}"] = np.ascontiguousarray(
                Wn[j * 128 : (j + 1) * 128, :]
            ).astype(ml_dtypes.bfloat16)
        in_maps.append(im)

    if cfg.get("SIM"):
        from concourse.bass_interp import MultiCoreSim

        sim = MultiCoreSim(nc, num_cores=C)
        for k, core_sim in sim.cores.items():
            for name, val in in_maps[k].items():
                core_sim.tensor(name)[:] = val
        sim.simulate()
        y = np.empty((N, OUT), np.float32)
        for k in range(C):
            y[k * NPC : (k + 1) * NPC] = sim.cores[k].tensor("ypad")[:NPC]
        return y

    global LAST_NC, LAST_RUN_S
    LAST_NC = nc
    import time as _time

    _t0 = _time.time()
    res = bass_utils.run_bass_kernel_spmd(
        nc,
        in_maps,
        core_ids=list(range(C)),
        trace=cfg.get("TRACE", False),
    )
    LAST_RUN_S = _time.time() - _t0
    LAST_RESULTS = res

    y = np.empty((N, OUT), np.float32)
    for k in range(C):
        y[k * NPC : (k + 1) * NPC] = res.results[k]["ypad"][:NPC]
    return y



# revision 2
# speedup vs baseline: 2.6836x; 2.6836x over previous
"""GCN block (GraphConv + BatchNorm1d + ReLU) on 8 Trainium2 NeuronCores.

Strategy: partition dst nodes across the 8 cores; every core keeps the FULL
x table (an external input, so it is staged for free) in its HBM and gathers
x[src] rows directly — the weight is applied AFTER aggregation, which is
exact because aggregation is linear:

    agg[d] = sum_e  nsrc[src_e] * ndst[d] * x[src_e]        (segmented sum)
    y_pre[d] = agg[d] @ W + b ; h = relu(y_pre) ; BN(h)

This removes the h AllGather of the previous design entirely (it cost
~700us of collective time on the critical path).

Per core k (owns dst nodes [k*NPC, (k+1)*NPC)), feature-major layout
([feature, node] on chip so bias/BN-affine are per-partition ACT ops):

  1. For each 128-edge block, gather x[src] rows (bf16, 512B/row -> full
     DMA-bus rate) and segment-sum with one-hot matmuls accumulated in PSUM.
     Both degree norms are folded into the one-hot matrix M as a per-edge
     scalar (dual-op tensor_scalar: is_equal then mult) at zero extra cost.
  2. Groups are processed in chunks of GC=3; each group owns a dedicated
     PSUM-bank pair that accumulates across all 4 src banks of the chunk
     (PSUM accumulation chains are bank-granular: 6 seg banks + 2 W banks).
  3. On group completion: agg pair -> SBUF bf16, W matmul -> [OUT, 128d]
     PSUM, relu(+bias ptr) with ACT accumulator producing BN sums on the
     fly; Square pass produces sum-of-squares.
  4. AllReduce the [128,2] BN sums; finalize scale/shift [128,1]; apply as
     per-partition ACT/DVE ops; write y^T to HBM (host transposes back).

Host-side work is limited to integer index bookkeeping (bucketing edges by
(core, chunk, src-bank, group), degree counting) and layout/dtype
transforms. All floating-point math runs on device.

Edges are bucketed by src bank (4 banks of 25000 rows) because dma_gather
indices are int16 (< 32768). Bucket sizes are padded to a structure shared
by all 8 cores so a single SPMD NEFF serves every core; pad slots gather
row 0 of the bank and carry a dst offset of 255 -> their one-hot column is
all zeros, so they contribute exactly 0.
"""
import math
import os
import sys

sys.path.insert(0, "/opt/trn_rl_repo")

import numpy as np

import concourse.bacc as bacc
import concourse.bass as bass
import concourse.mybir as mybir
import concourse.tile as tile
from concourse import bass_utils

F32 = mybir.dt.float32
BF16 = mybir.dt.bfloat16
I16 = mybir.dt.int16

CFG = dict(
    N=100000,
    E=1600000,
    IN=256,
    OUT=128,
    NCORES=8,
    GRP=128,          # dst nodes per segment group (= one-hot free dim)
    NBANKS=4,         # src banks (bank rows must stay < 32768 for int16 idx)
    XB=25000,         # rows per x bank
    GC=3,             # groups per chunk (2*GC psum seg banks + 2 W banks <= 8)
    BATCH_BLOCKS=48,  # gather batch cap in 128-edge blocks
    EPS=1e-5,
    TRACE=False,
)

LAST_RESULTS = None  # set by kernel() for test harness introspection
LAST_NC = None
LAST_RUN_S = None


def _ceil_div(a, b):
    return (a + b - 1) // b


def _wrap16(idx, ncols):
    """int16 idx list -> [128, ncols] tile: idx i at [i%16, i//16], replicated
    8x across the 16-partition groups (one copy per GpSimd Q7 core)."""
    n = idx.shape[0]
    assert n == ncols * 16
    w = np.ascontiguousarray(idx.reshape(ncols, 16).T)
    return np.tile(w, (8, 1))


def _preprocess(cfg, src, dst):
    """Bucket edges by (owner core, chunk, src bank, group); build per-core
    gather-index / dst-offset / per-edge-degree arrays and the shared static
    block structure."""
    N, E = cfg["N"], cfg["E"]
    C, NBANKS, GRP, GC = cfg["NCORES"], cfg["NBANKS"], cfg["GRP"], cfg["GC"]
    XB = cfg["XB"]
    NPC = N // C
    NG = _ceil_div(NPC, GRP)
    NCH = _ceil_div(NG, GC)
    assert XB * NBANKS == N and XB < 32768

    src = src.astype(np.int64)
    dst = dst.astype(np.int64)
    deg_out = np.bincount(src, minlength=N).astype(np.float32)
    deg_in = np.bincount(dst, minlength=N).astype(np.float32)

    owner = dst // NPC
    g_of = (dst % NPC) // GRP
    ch_of = g_of // GC
    gi_of = g_of - ch_of * GC
    bank = src // XB
    key = ((owner * NCH + ch_of) * NBANKS + bank) * GC + gi_of
    order = np.argsort(key, kind="stable")
    s_src = src[order]
    s_dst = dst[order]
    s_key = key[order]

    nkey = C * NCH * NBANKS * GC
    counts = np.bincount(key, minlength=nkey).reshape(C, NCH, NBANKS, GC)
    cmax = counts.max(axis=0)  # [NCH, NBANKS, GC]
    P = ((cmax + 127) // 128) * 128
    for ch in range(NCH):
        ngr = min(GC, NG - ch * GC)
        P[ch, 0, :ngr] = np.maximum(P[ch, 0, :ngr], 128)  # bank-0 run exists
        P[ch, :, ngr:] = 0

    # stream order: chunk outer, bank middle, group inner
    run_seq = []
    for ch in range(NCH):
        ngr = min(GC, NG - ch * GC)
        for b in range(NBANKS):
            for gi in range(ngr):
                if P[ch, b, gi] > 0:
                    run_seq.append((ch, b, gi))
    run_off = {}
    pos = 0
    for r in run_seq:
        run_off[r] = pos
        pos += int(P[r[0], r[1], r[2]])
    nidx_tot = pos
    nb_tot = nidx_tot // 128

    # last bank with edges per (ch, gi) -> is_stop position
    b_last = {}
    for ch in range(NCH):
        ngr = min(GC, NG - ch * GC)
        for gi in range(ngr):
            bl = 0
            for b in range(NBANKS):
                if P[ch, b, gi] > 0:
                    bl = b
            b_last[(ch, gi)] = bl

    # static block structure: per block t -> (g, gi, is_start, is_stop)
    blocks = []
    for ch, b, gi in run_seq:
        nb = int(P[ch, b, gi]) // 128
        g = ch * GC + gi
        for j in range(nb):
            is_start = b == 0 and j == 0
            is_stop = b == b_last[(ch, gi)] and j == nb - 1
            blocks.append((g, gi, is_start, is_stop))

    # gather batches: consecutive blocks within one (chunk, bank) stream
    batches = []  # (bank, first_block, n_blocks)
    t = 0
    for ch in range(NCH):
        ngr = min(GC, NG - ch * GC)
        for b in range(NBANKS):
            nb_cb = sum(int(P[ch, b, gi]) for gi in range(ngr)) // 128
            while nb_cb > 0:
                nb = min(nb_cb, cfg["BATCH_BLOCKS"])
                batches.append((b, t, nb))
                t += nb
                nb_cb -= nb
    assert t == nb_tot

    # boundaries of each (k, ch, b, gi) bucket in the sorted edge stream
    bkeys = np.arange(nkey)
    starts = np.searchsorted(s_key, bkeys).reshape(C, NCH, NBANKS, GC)
    ends = np.searchsorted(s_key, bkeys, side="right").reshape(C, NCH, NBANKS, GC)

    gidx_cores = []
    doff_cores = []
    dgo_cores = []
    dgi_cores = []
    for k in range(C):
        gidx = np.zeros(nidx_tot, np.int16)
        doff = np.full(nidx_tot, 255.0, np.float32)
        dgo = np.ones(nidx_tot, np.float32)
        dgi = np.ones(nidx_tot, np.float32)
        for ch, b, gi in run_seq:
            s, e = starts[k, ch, b, gi], ends[k, ch, b, gi]
            cnt = e - s
            if cnt == 0:
                continue
            p0 = run_off[(ch, b, gi)]
            g = ch * GC + gi
            gidx[p0 : p0 + cnt] = (s_src[s:e] % XB).astype(np.int16)
            doff[p0 : p0 + cnt] = ((s_dst[s:e] % NPC) - g * GRP).astype(np.float32)
            dgo[p0 : p0 + cnt] = deg_out[s_src[s:e]]
            dgi[p0 : p0 + cnt] = deg_in[s_dst[s:e]]
        gidx_cores.append(_wrap16(gidx, nidx_tot // 16))
        doff_cores.append(np.ascontiguousarray(doff.reshape(nb_tot, 128).T))
        dgo_cores.append(np.ascontiguousarray(dgo.reshape(nb_tot, 128).T))
        dgi_cores.append(np.ascontiguousarray(dgi.reshape(nb_tot, 128).T))

    meta = dict(
        NPC=NPC,
        NG=NG,
        NCH=NCH,
        nidx_tot=nidx_tot,
        nb_tot=nb_tot,
        blocks=blocks,
        batches=batches,
    )
    return meta, gidx_cores, doff_cores, dgo_cores, dgi_cores


def _build_nc(cfg, meta):
    N, IN, OUT, C = cfg["N"], cfg["IN"], cfg["OUT"], cfg["NCORES"]
    GRP, NBANKS, XB, GC = cfg["GRP"], cfg["NBANKS"], cfg["XB"], cfg["GC"]
    NPC, NG = meta["NPC"], meta["NG"]
    nidx_tot, nb_tot = meta["nidx_tot"], meta["nb_tot"]
    blocks, batches = meta["blocks"], meta["batches"]
    XK = _ceil_div(IN, 128)
    assert OUT == 128 and GRP == 128 and IN == 256
    last_w = NPC - (NG - 1) * GRP  # valid dst cols in the last group

    nc = bacc.Bacc(
        "TRN2", target_bir_lowering=False, debug=False, num_devices=C
    )

    # ---- external inputs ----
    xb = [
        nc.dram_tensor(f"xb{q}", [XB, IN], BF16, kind="ExternalInput")
        for q in range(NBANKS)
    ]
    wt = [
        nc.dram_tensor(f"wt{j}", [128, OUT], BF16, kind="ExternalInput")
        for j in range(XK)
    ]
    gidx_d = nc.dram_tensor("gidx", [128, nidx_tot // 16], I16, kind="ExternalInput")
    doff_d = nc.dram_tensor("doff", [128, nb_tot], F32, kind="ExternalInput")
    dgo_d = nc.dram_tensor("dgo", [128, nb_tot], F32, kind="ExternalInput")
    dgi_d = nc.dram_tensor("dgi", [128, nb_tot], F32, kind="ExternalInput")
    iota_d = nc.dram_tensor("iota", [128, GRP], BF16, kind="ExternalInput")
    btc_d = nc.dram_tensor("btc", [OUT, 1], F32, kind="ExternalInput")
    gmc_d = nc.dram_tensor("gmc", [OUT, 1], F32, kind="ExternalInput")
    bbc_d = nc.dram_tensor("bbc", [OUT, 1], F32, kind="ExternalInput")

    ypadT_d = nc.dram_tensor("ypadT", [OUT, NG * GRP], F32, kind="ExternalOutput")

    bmax = max(nb for _, _, nb in batches)

    with tile.TileContext(nc) as tc:
        with (
            tc.tile_pool(name="const", bufs=1) as cpool,
            tc.tile_pool(name="dram", bufs=1, space="DRAM") as dpool,
            tc.tile_pool(name="agg", bufs=1) as apool,
            tc.tile_pool(name="gath", bufs=3) as gpool,
            tc.tile_pool(name="mpool", bufs=6) as mpool,
            tc.tile_pool(name="asb", bufs=2) as asbp,
            tc.tile_pool(name="sq", bufs=2) as sqp,
            tc.tile_pool(name="pseg", bufs=1, space="PSUM") as psegp,
            tc.tile_pool(name="pw", bufs=2, space="PSUM") as pwp,
        ):
            # ---- constants / small tiles ----
            iota_t = cpool.tile([128, GRP], BF16)
            btc_t = cpool.tile([OUT, 1], F32)
            gmc_t = cpool.tile([OUT, 1], F32)
            bbc_t = cpool.tile([OUT, 1], F32)
            gidx_t = cpool.tile([128, nidx_tot // 16], I16)
            doff_t = cpool.tile([128, nb_tot], F32)
            dgo_t = cpool.tile([128, nb_tot], F32)
            dgi_t = cpool.tile([128, nb_tot], F32)
            stats_s = cpool.tile([OUT, NG], F32)
            stats_q = cpool.tile([OUT, NG], F32)
            wts = []
            for j in range(XK):
                wtile = cpool.tile([128, OUT], BF16, name=f"wt_s{j}")
                nc.sync.dma_start(wtile[:], wt[j][:])
                wts.append(wtile)

            nc.sync.dma_start(gidx_t[:], gidx_d[:])
            nc.sync.dma_start(doff_t[:], doff_d[:])
            nc.sync.dma_start(dgo_t[:], dgo_d[:])
            nc.sync.dma_start(dgi_t[:], dgi_d[:])
            nc.sync.dma_start(iota_t[:], iota_d[:])
            nc.sync.dma_start(btc_t[:], btc_d[:])
            nc.sync.dma_start(gmc_t[:], gmc_d[:])
            nc.sync.dma_start(bbc_t[:], bbc_d[:])

            # per-edge norm scale s = rsqrt(max(dgo,1)) * rsqrt(max(dgi,1))
            # (computed in-place in dgo_t; dgi_t is scratch after this)
            for deg_t in (dgo_t, dgi_t):
                nc.vector.tensor_scalar(
                    deg_t[:], deg_t[:], 1.0, None, op0=mybir.AluOpType.max
                )
                nc.vector.reciprocal(deg_t[:], deg_t[:])
                nc.scalar.activation(
                    deg_t[:], deg_t[:], mybir.ActivationFunctionType.Sqrt
                )
            s_t = dgo_t
            nc.vector.tensor_mul(s_t[:], dgo_t[:], dgi_t[:])

            # h table (feature-major): agg_t[:, g, d] = h[o, g*128+d]
            agg_t = apool.tile([OUT, NG, GRP], F32)
            # zero the last group's pad columns (stats square-pass reads them)
            nc.gpsimd.memset(agg_t[:, NG - 1, :], 0.0)

            # internal DRAM for the BN-stats collective
            stats_in = dpool.tile([OUT, 2], F32)
            stats_out = dpool.tile([OUT, 2], F32, addr_space="Shared")

            # ---- main loop: gather + one-hot matmul segmented sum ----
            cur_ps = {}  # gi -> (psA, psB)
            for bank, t0, nblk in batches:
                Gt = gpool.tile([128, bmax, IN], BF16, tag="G")
                nc.gpsimd.dma_gather(
                    Gt[:, :nblk, :],
                    xb[bank][:],
                    gidx_t[:, t0 * 8 : (t0 + nblk) * 8],
                    nblk * 128,
                    nblk * 128,
                    IN,
                    single_packet=False,
                )
                for j in range(nblk):
                    t = t0 + j
                    g, gi, is_start, is_stop = blocks[t]
                    Mt = mpool.tile([128, GRP], BF16, tag="M")
                    nc.vector.tensor_scalar(
                        Mt[:],
                        iota_t[:],
                        doff_t[:, t : t + 1],
                        s_t[:, t : t + 1],
                        op0=mybir.AluOpType.is_equal,
                        op1=mybir.AluOpType.mult,
                    )
                    if is_start:
                        psA = psegp.tile([128, GRP], F32, tag=f"sA{gi}", name=f"psA{gi}")
                        psB = psegp.tile([128, GRP], F32, tag=f"sB{gi}", name=f"psB{gi}")
                        cur_ps[gi] = (psA, psB)
                    psA, psB = cur_ps[gi]
                    nc.tensor.matmul(
                        psA[:], Gt[:, j, 0:128], Mt[:], start=is_start, stop=is_stop
                    )
                    nc.tensor.matmul(
                        psB[:], Gt[:, j, 128:256], Mt[:], start=is_start, stop=is_stop
                    )
                    if is_stop:
                        aggA = asbp.tile([128, GRP], BF16, tag="aggA")
                        aggB = asbp.tile([128, GRP], BF16, tag="aggB")
                        nc.scalar.activation(
                            aggA[:], psA[:], mybir.ActivationFunctionType.Copy
                        )
                        nc.scalar.activation(
                            aggB[:], psB[:], mybir.ActivationFunctionType.Copy
                        )
                        pso = pwp.tile([OUT, GRP], F32, tag="w")
                        nc.tensor.matmul(
                            pso[:], wts[0][:], aggA[:], start=True, stop=False
                        )
                        nc.tensor.matmul(
                            pso[:], wts[1][:], aggB[:], start=False, stop=True
                        )
                        w = GRP if g < NG - 1 else last_w
                        nc.scalar.activation(
                            agg_t[:, g, :w],
                            pso[:, :w],
                            mybir.ActivationFunctionType.Relu,
                            bias=btc_t[:, 0:1],
                            accum_out=stats_s[:, g : g + 1],
                        )
                        sqt = sqp.tile([OUT, GRP], F32, tag="sq")
                        nc.scalar.activation(
                            sqt[:, :w],
                            agg_t[:, g, :w],
                            mybir.ActivationFunctionType.Square,
                            accum_out=stats_q[:, g : g + 1],
                        )

            # ---- BN stats AllReduce + affine finalize ----
            stsb = cpool.tile([OUT, 2], F32)
            nc.vector.tensor_reduce(
                stsb[:, 0:1], stats_s[:], mybir.AxisListType.X, mybir.AluOpType.add
            )
            nc.vector.tensor_reduce(
                stsb[:, 1:2], stats_q[:], mybir.AxisListType.X, mybir.AluOpType.add
            )
            nc.sync.dma_start(stats_in[:], stsb[:])
            nc.gpsimd.collective_compute(
                "AllReduce",
                mybir.AluOpType.add,
                replica_groups=[list(range(C))],
                ins=[stats_in[:]],
                outs=[stats_out[:]],
            )
            strb = cpool.tile([OUT, 2], F32)
            nc.sync.dma_start(strb[:], stats_out[:])

            mu = cpool.tile([OUT, 1], F32)
            ex2 = cpool.tile([OUT, 1], F32)
            var = cpool.tile([OUT, 1], F32)
            S_t = cpool.tile([OUT, 1], F32)
            T_t = cpool.tile([OUT, 1], F32)
            inv_n = 1.0 / float(N)
            nc.scalar.activation(
                mu[:], strb[:, 0:1], mybir.ActivationFunctionType.Copy, scale=inv_n
            )
            nc.scalar.activation(
                ex2[:], strb[:, 1:2], mybir.ActivationFunctionType.Copy, scale=inv_n
            )
            nc.scalar.activation(var[:], mu[:], mybir.ActivationFunctionType.Square)
            nc.vector.tensor_sub(var[:], ex2[:], var[:])
            # var <- rsqrt(var + eps) (ACT Rsqrt is banned for accuracy)
            nc.scalar.activation(
                var[:], var[:], mybir.ActivationFunctionType.Copy,
                bias=float(cfg["EPS"]),
            )
            nc.vector.reciprocal(var[:], var[:])
            nc.scalar.activation(var[:], var[:], mybir.ActivationFunctionType.Sqrt)
            nc.vector.tensor_mul(S_t[:], gmc_t[:], var[:])
            nc.vector.tensor_mul(T_t[:], mu[:], S_t[:])
            nc.vector.tensor_sub(T_t[:], bbc_t[:], T_t[:])

            # ---- apply affine: y = h*S + T (ACT and DVE split halves) ----
            NH = NG // 2
            nc.scalar.activation(
                agg_t[:, :NH, :],
                agg_t[:, :NH, :],
                mybir.ActivationFunctionType.Identity,
                bias=T_t[:, 0:1],
                scale=S_t[:, 0:1],
            )
            nc.vector.tensor_scalar(
                agg_t[:, NH:, :],
                agg_t[:, NH:, :],
                S_t[:, 0:1],
                T_t[:, 0:1],
                op0=mybir.AluOpType.mult,
                op1=mybir.AluOpType.add,
            )
            ypadT_view = ypadT_d[:].rearrange("p (g f) -> p g f", g=NG)
            nc.sync.dma_start(ypadT_view[:, :NH, :], agg_t[:, :NH, :])
            nc.sync.dma_start(ypadT_view[:, NH:, :], agg_t[:, NH:, :])

    nc.compile()
    return nc


def kernel(x, src, dst, W, b, gamma, beta):
    global LAST_RESULTS
    cfg = CFG
    N, E, IN, OUT, C = cfg["N"], cfg["E"], cfg["IN"], cfg["OUT"], cfg["NCORES"]
    GRP, XB, NBANKS = cfg["GRP"], cfg["XB"], cfg["NBANKS"]
    assert x.shape == (N, IN) and W.shape == (IN, OUT)
    assert src.shape == (E,) and dst.shape == (E,)

    meta, gidx_cores, doff_cores, dgo_cores, dgi_cores = _preprocess(cfg, src, dst)
    NPC, NG = meta["NPC"], meta["NG"]
    XK = _ceil_div(IN, 128)

    nc = _build_nc(cfg, meta)

    import ml_dtypes

    xbf = np.asarray(x, np.float32).astype(ml_dtypes.bfloat16)  # [N, IN]
    Wn = np.asarray(W, np.float32)

    iota = np.tile(
        np.arange(GRP, dtype=np.float32)[None, :], (128, 1)
    ).astype(ml_dtypes.bfloat16)
    btc = np.ascontiguousarray(np.asarray(b, np.float32)[:, None])
    gmc = np.ascontiguousarray(np.asarray(gamma, np.float32)[:, None])
    bbc = np.ascontiguousarray(np.asarray(beta, np.float32)[:, None])

    xbanks = {
        f"xb{q}": np.ascontiguousarray(xbf[q * XB : (q + 1) * XB, :])
        for q in range(NBANKS)
    }
    wmap = {
        f"wt{j}": np.ascontiguousarray(
            Wn[j * 128 : (j + 1) * 128, :]
        ).astype(ml_dtypes.bfloat16)
        for j in range(XK)
    }

    in_maps = []
    for k in range(C):
        im = {
            "gidx": gidx_cores[k],
            "doff": doff_cores[k],
            "dgo": dgo_cores[k],
            "dgi": dgi_cores[k],
            "iota": iota,
            "btc": btc,
            "gmc": gmc,
            "bbc": bbc,
        }
        im.update(xbanks)
        im.update(wmap)
        in_maps.append(im)

    if cfg.get("SIM"):
        from concourse.bass_interp import MultiCoreSim

        sim = MultiCoreSim(nc, num_cores=C)
        for k, core_sim in sim.cores.items():
            for name, val in in_maps[k].items():
                core_sim.tensor(name)[:] = val
        sim.simulate()
        y = np.empty((N, OUT), np.float32)
        for k in range(C):
            y[k * NPC : (k + 1) * NPC] = sim.cores[k].tensor("ypadT")[:, :NPC].T
        return y

    global LAST_NC, LAST_RUN_S
    LAST_NC = nc
    import time as _time

    _t0 = _time.time()
    res = bass_utils.run_bass_kernel_spmd(
        nc,
        in_maps,
        core_ids=list(range(C)),
        trace=cfg.get("TRACE", False),
    )
    LAST_RUN_S = _time.time() - _t0
    LAST_RESULTS = res

    y = np.empty((N, OUT), np.float32)
    for k in range(C):
        y[k * NPC : (k + 1) * NPC] = res.results[k]["ypadT"][:, :NPC].T
    return y


# revision 5
# speedup vs baseline: 2.8467x; 1.0608x over previous
"""GCN block (GraphConv + BatchNorm1d + ReLU) on 8 Trainium2 NeuronCores.

Strategy: partition dst nodes across the 8 cores; every core keeps the FULL
x table (an external input, so it is staged for free) in its HBM and gathers
x[src] rows directly — the weight is applied AFTER aggregation, which is
exact because aggregation is linear:

    agg[d] = sum_e  nsrc[src_e] * ndst[d] * x[src_e]        (segmented sum)
    y_pre[d] = agg[d] @ W + b ; h = relu(y_pre) ; BN(h)

This removes the h AllGather of the original design entirely (it cost
~700us of collective time on the critical path).

Layout is feature-major on chip ([feature, node]) so bias and the BN affine
are per-partition ACT ops, and BN batch sums fall out of the ACT
accumulator for free.

The dst->core assignment is ours to choose, so nodes are assigned to
(core, group) slots by a degree-profile-aware round-robin (nodes with equal
per-bank in-degree profiles are dealt cyclically across all 784 slots).
This equalizes every (group, bank) bucket's edge count across the 8 cores,
collapsing the shared-NEFF padding slack from ~11% to ~2%; buckets are then
padded to 16-slot granularity (the dma_gather index wrap).  128-edge blocks
may straddle group boundaries: each (block, group) pair in the shared
schedule gets its own one-hot matrix M built from a per-pair pre-shifted
dst-offset column (values outside [0,128) give zero columns, so foreign
and pad slots contribute exactly 0).  Per-edge degree norms are folded into
M by the dual-op tensor_scalar (is_equal then mult) at zero extra cost.

Groups are processed in chunks of GC=3: each group owns a dedicated
PSUM-bank pair whose accumulation chain spans all 4 src banks of its chunk
(6 seg banks + 2 W banks = all 8).  On group completion: agg pair -> SBUF
bf16, W matmul, relu(+bias ptr) with ACT accumulator emitting BN sums,
Square pass emitting sum-of-squares; AllReduce [128,2]; per-partition
affine; y^T written to HBM (host permutes rows back).

Host-side work is limited to integer index bookkeeping and layout/dtype
transforms. All floating-point math runs on device.
"""
import math
import os
import sys

sys.path.insert(0, "/opt/trn_rl_repo")

import numpy as np

import concourse.bacc as bacc
import concourse.bass as bass
import concourse.mybir as mybir
import concourse.tile as tile
from concourse import bass_utils

F32 = mybir.dt.float32
BF16 = mybir.dt.bfloat16
I16 = mybir.dt.int16

CFG = dict(
    N=100000,
    E=1600000,
    IN=256,
    OUT=128,
    NCORES=8,
    GRP=128,          # dst nodes per segment group (= one-hot free dim)
    NBANKS=4,         # src banks (bank rows must stay < 32768 for int16 idx)
    XB=25000,         # rows per x bank
    GC=3,             # groups per chunk (2*GC psum seg banks + 2 W banks <= 8)
    BATCH_BLOCKS=48,  # gather batch cap in 128-edge blocks
    EPS=1e-5,
    TRACE=False,
)

LAST_RESULTS = None  # set by kernel() for test harness introspection
LAST_NC = None
LAST_RUN_S = None


def _ceil_div(a, b):
    return (a + b - 1) // b


def _wrap16(idx, ncols):
    """int16 idx list -> [128, ncols] tile: idx i at [i%16, i//16], replicated
    8x across the 16-partition groups (one copy per GpSimd Q7 core)."""
    n = idx.shape[0]
    assert n == ncols * 16
    w = np.ascontiguousarray(idx.reshape(ncols, 16).T)
    return np.tile(w, (8, 1))


def _balance_nodes(cfg, src, dst):
    """Assign dst nodes to (core, group) slots so that every (group-pos,
    bank) bucket has a near-equal edge count on all 8 cores.  Nodes with
    identical per-bank in-degree profiles are dealt round-robin across all
    slots.  Returns newpos[node] (position in the concatenated core
    layout)."""
    N = cfg["N"]
    C, NG, GRP, XB = cfg["NCORES"], _ceil_div(N // cfg["NCORES"], cfg["GRP"]), cfg["GRP"], cfg["XB"]
    NPC = N // C
    NS = C * NG
    last_w = NPC - (NG - 1) * GRP

    bank_e = src // XB
    prof = np.bincount(dst * 4 + bank_e, minlength=N * 4).reshape(N, 4)
    _, inv = np.unique(prof, axis=0, return_inverse=True)
    order_nodes = np.argsort(inv, kind="stable")
    cls_sorted = inv[order_nodes]
    seg = np.flatnonzero(np.diff(cls_sorted)) + 1
    seg_starts = np.concatenate([[0], seg, [N]])

    slot_of = np.empty(N, np.int64)
    ptr = 0
    for i in range(len(seg_starts) - 1):
        a, b = seg_starts[i], seg_starts[i + 1]
        m = b - a
        slot_of[order_nodes[a:b]] = (np.arange(m) + ptr) % NS
        ptr = (ptr + m) % NS
    # capacity fix: slots (k, NG-1) hold only last_w nodes
    cap = np.full(NS, GRP, np.int64)
    cap[(NG - 1) * C :] = last_w  # slot id s: gp = s // C, core = s % C
    by_slot = np.argsort(slot_of, kind="stable")
    fill = np.bincount(slot_of, minlength=NS)
    cum = np.concatenate([[0], np.cumsum(fill)])
    moved = []
    for s in range(NS):
        if fill[s] > cap[s]:
            moved.extend(by_slot[cum[s] + cap[s] : cum[s + 1]])
    if moved:
        room_slots = np.repeat(
            np.arange(NS), np.maximum(cap - fill, 0)
        )[: len(moved)]
        slot_of[np.array(moved)] = room_slots
    # final positions
    by_slot = np.argsort(slot_of, kind="stable")
    fill = np.bincount(slot_of, minlength=NS)
    assert (fill == cap).all()
    offs = np.arange(N) - np.repeat(
        np.concatenate([[0], np.cumsum(fill)])[:-1], fill
    )
    s_sorted = slot_of[by_slot]
    newpos = np.empty(N, np.int64)
    newpos[by_slot] = (s_sorted % C) * NPC + (s_sorted // C) * GRP + offs
    return newpos


def _preprocess(cfg, src, dst):
    """Bucket edges by (owner core, chunk, src bank, group) under the
    balanced node assignment; build the shared (block, group) pair schedule
    and per-core gather-index / dst-offset / per-edge-degree arrays."""
    N, E = cfg["N"], cfg["E"]
    C, NBANKS, GRP, GC = cfg["NCORES"], cfg["NBANKS"], cfg["GRP"], cfg["GC"]
    XB = cfg["XB"]
    NPC = N // C
    NG = _ceil_div(NPC, GRP)
    NCH = _ceil_div(NG, GC)
    assert XB * NBANKS == N and XB < 32768

    src = src.astype(np.int64)
    dst = dst.astype(np.int64)
    deg_out = np.bincount(src, minlength=N).astype(np.float32)
    deg_in = np.bincount(dst, minlength=N).astype(np.float32)

    newpos = _balance_nodes(cfg, src, dst)
    dstN = newpos[dst]

    owner = dstN // NPC
    g_of = (dstN % NPC) // GRP
    ch_of = g_of // GC
    gi_of = g_of - ch_of * GC
    bank = src // XB
    key = ((owner * NCH + ch_of) * NBANKS + bank) * GC + gi_of
    order = np.argsort(key, kind="stable")
    s_src = src[order]
    s_dstN = dstN[order]
    s_dstO = dst[order]
    s_key = key[order]

    nkey = C * NCH * NBANKS * GC
    counts = np.bincount(key, minlength=nkey).reshape(C, NCH, NBANKS, GC)
    cmax = counts.max(axis=0)  # [NCH, NBANKS, GC]
    R = ((cmax + 15) // 16) * 16
    for ch in range(NCH):
        ngr = min(GC, NG - ch * GC)
        R[ch, 0, :ngr] = np.maximum(R[ch, 0, :ngr], 16)  # bank-0 run exists
        R[ch, :, ngr:] = 0

    # stream layout: per (chunk, bank): runs at 16-slot granularity, stream
    # rounded up to whole 128-slot blocks
    run_off = {}
    stream_blk0 = {}
    pos = 0
    for ch in range(NCH):
        ngr = min(GC, NG - ch * GC)
        for b in range(NBANKS):
            assert pos % 128 == 0
            stream_blk0[(ch, b)] = pos // 128
            for gi in range(ngr):
                if R[ch, b, gi] > 0:
                    run_off[(ch, b, gi)] = pos
                    pos += int(R[ch, b, gi])
            pos = _ceil_div(pos, 128) * 128
    nidx_tot = pos
    nb_tot = nidx_tot // 128

    # (block, group) pair schedule + per-group chain flags
    pairs = []           # (t, g)
    group_pairs = {}     # g -> [pair indices in emission order]
    block_pairs = [[] for _ in range(nb_tot)]
    for ch in range(NCH):
        ngr = min(GC, NG - ch * GC)
        for b in range(NBANKS):
            for gi in range(ngr):
                if R[ch, b, gi] == 0:
                    continue
                g = ch * GC + gi
                r0 = run_off[(ch, b, gi)]
                r1 = r0 + int(R[ch, b, gi])
                for t in range(r0 // 128, (r1 - 1) // 128 + 1):
                    p = len(pairs)
                    pairs.append((t, g))
                    group_pairs.setdefault(g, []).append(p)
                    block_pairs[t].append(p)
    npairs = len(pairs)
    pair_info = []
    starts_set = {gp[0] for gp in group_pairs.values()}
    stops_set = {gp[-1] for gp in group_pairs.values()}
    for p, (t, g) in enumerate(pairs):
        pair_info.append((g, p in starts_set, p in stops_set))
    # dedupe: a straddling run can emit two pairs (t, g) for consecutive
    # runs of the same g in different banks mapping to the same t — they
    # are distinct pairs (per-bank), which is fine for the psum chain.

    # gather batches: consecutive blocks within one (chunk, bank) stream
    batches = []  # (bank, first_block, n_blocks)
    for ch in range(NCH):
        for b in range(NBANKS):
            t0 = stream_blk0[(ch, b)]
            t1 = stream_blk0.get((ch, b + 1))
            if t1 is None:
                t1 = stream_blk0.get((ch + 1, 0), nb_tot)
            rem = t1 - t0
            t = t0
            while rem > 0:
                nb = min(rem, cfg["BATCH_BLOCKS"])
                batches.append((b, t, nb))
                t += nb
                rem -= nb

    # per (k, ch, b, gi) boundaries in the sorted edge stream
    bkeys = np.arange(nkey)
    bstarts = np.searchsorted(s_key, bkeys).reshape(C, NCH, NBANKS, GC)
    bends = np.searchsorted(s_key, bkeys, side="right").reshape(C, NCH, NBANKS, GC)

    gidx_cores = []
    doff_cores = []
    dgo_cores = []
    dgi_cores = []
    for k in range(C):
        gidx = np.zeros(nidx_tot, np.int16)
        dmod = np.full(nidx_tot, -1.0e6, np.float32)
        dgo = np.ones(nidx_tot, np.float32)
        dgi = np.ones(nidx_tot, np.float32)
        for (ch, b, gi), p0 in run_off.items():
            s, e = int(bstarts[k, ch, b, gi]), int(bends[k, ch, b, gi])
            cnt = e - s
            if cnt == 0:
                continue
            gidx[p0 : p0 + cnt] = (s_src[s:e] % XB).astype(np.int16)
            dmod[p0 : p0 + cnt] = (s_dstN[s:e] % NPC).astype(np.float32)
            dgo[p0 : p0 + cnt] = deg_out[s_src[s:e]]
            dgi[p0 : p0 + cnt] = deg_in[s_dstO[s:e]]
        dmod2 = dmod.reshape(nb_tot, 128)
        doff = np.empty((npairs, 128), np.float32)
        for p, (t, g) in enumerate(pairs):
            doff[p] = dmod2[t] - np.float32(g * GRP)
        gidx_cores.append(_wrap16(gidx, nidx_tot // 16))
        doff_cores.append(np.ascontiguousarray(doff.T))
        dgo_cores.append(np.ascontiguousarray(dgo.reshape(nb_tot, 128).T))
        dgi_cores.append(np.ascontiguousarray(dgi.reshape(nb_tot, 128).T))

    meta = dict(
        NPC=NPC,
        NG=NG,
        nidx_tot=nidx_tot,
        nb_tot=nb_tot,
        npairs=npairs,
        pairs=pairs,
        pair_info=pair_info,
        block_pairs=block_pairs,
        batches=batches,
        newpos=newpos,
    )
    return meta, gidx_cores, doff_cores, dgo_cores, dgi_cores


def _build_nc(cfg, meta):
    N, IN, OUT, C = cfg["N"], cfg["IN"], cfg["OUT"], cfg["NCORES"]
    GRP, NBANKS, XB, GC = cfg["GRP"], cfg["NBANKS"], cfg["XB"], cfg["GC"]
    NPC, NG = meta["NPC"], meta["NG"]
    nidx_tot, nb_tot = meta["nidx_tot"], meta["nb_tot"]
    npairs = meta["npairs"]
    pair_info = meta["pair_info"]
    block_pairs = meta["block_pairs"]
    batches = meta["batches"]
    XK = _ceil_div(IN, 128)
    assert OUT == 128 and GRP == 128 and IN == 256
    last_w = NPC - (NG - 1) * GRP  # valid dst cols in the last group

    nc = bacc.Bacc(
        "TRN2", target_bir_lowering=False, debug=False, num_devices=C
    )

    # ---- external inputs ----
    xb = [
        nc.dram_tensor(f"xb{q}", [XB, IN], BF16, kind="ExternalInput")
        for q in range(NBANKS)
    ]
    wt = [
        nc.dram_tensor(f"wt{j}", [128, OUT], BF16, kind="ExternalInput")
        for j in range(XK)
    ]
    gidx_d = nc.dram_tensor("gidx", [128, nidx_tot // 16], I16, kind="ExternalInput")
    doff_d = nc.dram_tensor("doff", [128, npairs], F32, kind="ExternalInput")
    dgo_d = nc.dram_tensor("dgo", [128, nb_tot], F32, kind="ExternalInput")
    dgi_d = nc.dram_tensor("dgi", [128, nb_tot], F32, kind="ExternalInput")
    iota_d = nc.dram_tensor("iota", [128, GRP], BF16, kind="ExternalInput")
    btc_d = nc.dram_tensor("btc", [OUT, 1], F32, kind="ExternalInput")
    gmc_d = nc.dram_tensor("gmc", [OUT, 1], F32, kind="ExternalInput")
    bbc_d = nc.dram_tensor("bbc", [OUT, 1], F32, kind="ExternalInput")

    ypadT_d = nc.dram_tensor("ypadT", [OUT, NG * GRP], F32, kind="ExternalOutput")

    bmax = max(nb for _, _, nb in batches)

    with tile.TileContext(nc) as tc:
        with (
            tc.tile_pool(name="const", bufs=1) as cpool,
            tc.tile_pool(name="dram", bufs=1, space="DRAM") as dpool,
            tc.tile_pool(name="agg", bufs=1) as apool,
            tc.tile_pool(name="gath", bufs=3) as gpool,
            tc.tile_pool(name="mpool", bufs=6) as mpool,
            tc.tile_pool(name="asb", bufs=2) as asbp,
            tc.tile_pool(name="sq", bufs=2) as sqp,
            tc.tile_pool(name="pseg", bufs=1, space="PSUM") as psegp,
            tc.tile_pool(name="pw", bufs=2, space="PSUM") as pwp,
        ):
            # ---- constants / small tiles ----
            iota_t = cpool.tile([128, GRP], BF16)
            btc_t = cpool.tile([OUT, 1], F32)
            gmc_t = cpool.tile([OUT, 1], F32)
            bbc_t = cpool.tile([OUT, 1], F32)
            gidx_t = cpool.tile([128, nidx_tot // 16], I16)
            doff_t = cpool.tile([128, npairs], F32)
            dgo_t = cpool.tile([128, nb_tot], F32)
            dgi_t = cpool.tile([128, nb_tot], F32)
            stats_s = cpool.tile([OUT, NG], F32)
            stats_q = cpool.tile([OUT, NG], F32)
            wts = []
            for j in range(XK):
                wtile = cpool.tile([128, OUT], BF16, name=f"wt_s{j}")
                nc.sync.dma_start(wtile[:], wt[j][:])
                wts.append(wtile)

            nc.sync.dma_start(gidx_t[:], gidx_d[:])
            nc.sync.dma_start(doff_t[:], doff_d[:])
            nc.sync.dma_start(dgo_t[:], dgo_d[:])
            nc.sync.dma_start(dgi_t[:], dgi_d[:])
            nc.sync.dma_start(iota_t[:], iota_d[:])
            nc.sync.dma_start(btc_t[:], btc_d[:])
            nc.sync.dma_start(gmc_t[:], gmc_d[:])
            nc.sync.dma_start(bbc_t[:], bbc_d[:])

            # per-edge norm scale s = rsqrt(max(dgo,1)) * rsqrt(max(dgi,1))
            # (computed in-place in dgo_t; dgi_t is scratch after this)
            for deg_t in (dgo_t, dgi_t):
                nc.vector.tensor_scalar(
                    deg_t[:], deg_t[:], 1.0, None, op0=mybir.AluOpType.max
                )
                nc.vector.reciprocal(deg_t[:], deg_t[:])
                nc.scalar.activation(
                    deg_t[:], deg_t[:], mybir.ActivationFunctionType.Sqrt
                )
            s_t = dgo_t
            nc.vector.tensor_mul(s_t[:], dgo_t[:], dgi_t[:])

            # h table (feature-major): agg_t[:, g, d] = h[o, g*128+d]
            agg_t = apool.tile([OUT, NG, GRP], F32)
            # zero the last group's pad columns (stats square-pass reads them)
            nc.gpsimd.memset(agg_t[:, NG - 1, :], 0.0)

            # internal DRAM for the BN-stats collective
            stats_in = dpool.tile([OUT, 2], F32)
            stats_out = dpool.tile([OUT, 2], F32, addr_space="Shared")

            # ---- main loop: gather + one-hot matmul segmented sum ----
            cur_ps = {}  # gi -> (psA, psB)
            for bank, t0, nblk in batches:
                Gt = gpool.tile([128, bmax, IN], BF16, tag="G")
                nc.gpsimd.dma_gather(
                    Gt[:, :nblk, :],
                    xb[bank][:],
                    gidx_t[:, t0 * 8 : (t0 + nblk) * 8],
                    nblk * 128,
                    nblk * 128,
                    IN,
                    single_packet=False,
                )
                for j in range(nblk):
                    t = t0 + j
                    for p in block_pairs[t]:
                        g, is_start, is_stop = pair_info[p]
                        gi = g % GC
                        Mt = mpool.tile([128, GRP], BF16, tag="M")
                        nc.vector.tensor_scalar(
                            Mt[:],
                            iota_t[:],
                            doff_t[:, p : p + 1],
                            s_t[:, t : t + 1],
                            op0=mybir.AluOpType.is_equal,
                            op1=mybir.AluOpType.mult,
                        )
                        if is_start:
                            psA = psegp.tile(
                                [128, GRP], F32, tag=f"sA{gi}", name=f"psA{gi}"
                            )
                            psB = psegp.tile(
                                [128, GRP], F32, tag=f"sB{gi}", name=f"psB{gi}"
                            )
                            cur_ps[gi] = (psA, psB)
                        psA, psB = cur_ps[gi]
                        nc.tensor.matmul(
                            psA[:], Gt[:, j, 0:128], Mt[:],
                            start=is_start, stop=is_stop,
                        )
                        nc.tensor.matmul(
                            psB[:], Gt[:, j, 128:256], Mt[:],
                            start=is_start, stop=is_stop,
                        )
                        if not is_stop:
                            continue
                        aggA = asbp.tile([128, GRP], BF16, tag="aggA")
                        aggB = asbp.tile([128, GRP], BF16, tag="aggB")
                        nc.scalar.activation(
                            aggA[:], psA[:], mybir.ActivationFunctionType.Copy
                        )
                        nc.scalar.activation(
                            aggB[:], psB[:], mybir.ActivationFunctionType.Copy
                        )
                        pso = pwp.tile([OUT, GRP], F32, tag="w")
                        nc.tensor.matmul(
                            pso[:], wts[0][:], aggA[:], start=True, stop=False
                        )
                        nc.tensor.matmul(
                            pso[:], wts[1][:], aggB[:], start=False, stop=True
                        )
                        w = GRP if g < NG - 1 else last_w
                        nc.scalar.activation(
                            agg_t[:, g, :w],
                            pso[:, :w],
                            mybir.ActivationFunctionType.Relu,
                            bias=btc_t[:, 0:1],
                            accum_out=stats_s[:, g : g + 1],
                        )
                        sqt = sqp.tile([OUT, GRP], F32, tag="sq")
                        nc.scalar.activation(
                            sqt[:, :w],
                            agg_t[:, g, :w],
                            mybir.ActivationFunctionType.Square,
                            accum_out=stats_q[:, g : g + 1],
                        )

            # ---- BN stats AllReduce + affine finalize ----
            stsb = cpool.tile([OUT, 2], F32)
            nc.vector.tensor_reduce(
                stsb[:, 0:1], stats_s[:], mybir.AxisListType.X, mybir.AluOpType.add
            )
            nc.vector.tensor_reduce(
                stsb[:, 1:2], stats_q[:], mybir.AxisListType.X, mybir.AluOpType.add
            )
            nc.sync.dma_start(stats_in[:], stsb[:])
            nc.gpsimd.collective_compute(
                "AllReduce",
                mybir.AluOpType.add,
                replica_groups=[list(range(C))],
                ins=[stats_in[:]],
                outs=[stats_out[:]],
            )
            strb = cpool.tile([OUT, 2], F32)
            nc.sync.dma_start(strb[:], stats_out[:])

            mu = cpool.tile([OUT, 1], F32)
            ex2 = cpool.tile([OUT, 1], F32)
            var = cpool.tile([OUT, 1], F32)
            S_t = cpool.tile([OUT, 1], F32)
            T_t = cpool.tile([OUT, 1], F32)
            inv_n = 1.0 / float(N)
            nc.scalar.activation(
                mu[:], strb[:, 0:1], mybir.ActivationFunctionType.Copy, scale=inv_n
            )
            nc.scalar.activation(
                ex2[:], strb[:, 1:2], mybir.ActivationFunctionType.Copy, scale=inv_n
            )
            nc.scalar.activation(var[:], mu[:], mybir.ActivationFunctionType.Square)
            nc.vector.tensor_sub(var[:], ex2[:], var[:])
            # var <- rsqrt(var + eps) (ACT Rsqrt is banned for accuracy)
            nc.scalar.activation(
                var[:], var[:], mybir.ActivationFunctionType.Copy,
                bias=float(cfg["EPS"]),
            )
            nc.vector.reciprocal(var[:], var[:])
            nc.scalar.activation(var[:], var[:], mybir.ActivationFunctionType.Sqrt)
            nc.vector.tensor_mul(S_t[:], gmc_t[:], var[:])
            nc.vector.tensor_mul(T_t[:], mu[:], S_t[:])
            nc.vector.tensor_sub(T_t[:], bbc_t[:], T_t[:])

            # ---- apply affine: y = h*S + T (ACT and DVE split halves) ----
            NH = NG // 2
            nc.scalar.activation(
                agg_t[:, :NH, :],
                agg_t[:, :NH, :],
                mybir.ActivationFunctionType.Identity,
                bias=T_t[:, 0:1],
                scale=S_t[:, 0:1],
            )
            nc.vector.tensor_scalar(
                agg_t[:, NH:, :],
                agg_t[:, NH:, :],
                S_t[:, 0:1],
                T_t[:, 0:1],
                op0=mybir.AluOpType.mult,
                op1=mybir.AluOpType.add,
            )
            ypadT_view = ypadT_d[:].rearrange("p (g f) -> p g f", g=NG)
            nc.sync.dma_start(ypadT_view[:, :NH, :], agg_t[:, :NH, :])
            nc.sync.dma_start(ypadT_view[:, NH:, :], agg_t[:, NH:, :])

    nc.compile()
    return nc


def kernel(x, src, dst, W, b, gamma, beta):
    global LAST_RESULTS
    cfg = CFG
    N, E, IN, OUT, C = cfg["N"], cfg["E"], cfg["IN"], cfg["OUT"], cfg["NCORES"]
    GRP, XB, NBANKS = cfg["GRP"], cfg["XB"], cfg["NBANKS"]
    assert x.shape == (N, IN) and W.shape == (IN, OUT)
    assert src.shape == (E,) and dst.shape == (E,)

    meta, gidx_cores, doff_cores, dgo_cores, dgi_cores = _preprocess(cfg, src, dst)
    NPC, NG = meta["NPC"], meta["NG"]
    newpos = meta["newpos"]
    XK = _ceil_div(IN, 128)

    nc = _build_nc(cfg, meta)

    import ml_dtypes

    xbf = np.asarray(x, np.float32).astype(ml_dtypes.bfloat16)  # [N, IN]
    Wn = np.asarray(W, np.float32)

    iota = np.tile(
        np.arange(GRP, dtype=np.float32)[None, :], (128, 1)
    ).astype(ml_dtypes.bfloat16)
    btc = np.ascontiguousarray(np.asarray(b, np.float32)[:, None])
    gmc = np.ascontiguousarray(np.asarray(gamma, np.float32)[:, None])
    bbc = np.ascontiguousarray(np.asarray(beta, np.float32)[:, None])

    xbanks = {
        f"xb{q}": np.ascontiguousarray(xbf[q * XB : (q + 1) * XB, :])
        for q in range(NBANKS)
    }
    wmap = {
        f"wt{j}": np.ascontiguousarray(
            Wn[j * 128 : (j + 1) * 128, :]
        ).astype(ml_dtypes.bfloat16)
        for j in range(XK)
    }

    in_maps = []
    for k in range(C):
        im = {
            "gidx": gidx_cores[k],
            "doff": doff_cores[k],
            "dgo": dgo_cores[k],
            "dgi": dgi_cores[k],
            "iota": iota,
            "btc": btc,
            "gmc": gmc,
            "bbc": bbc,
        }
        im.update(xbanks)
        im.update(wmap)
        in_maps.append(im)

    if cfg.get("SIM"):
        from concourse.bass_interp import MultiCoreSim

        sim = MultiCoreSim(nc, num_cores=C)
        for k, core_sim in sim.cores.items():
            for name, val in in_maps[k].items():
                core_sim.tensor(name)[:] = val
        sim.simulate()
        ycomp = np.empty((N, OUT), np.float32)
        for k in range(C):
            ycomp[k * NPC : (k + 1) * NPC] = sim.cores[k].tensor("ypadT")[:, :NPC].T
        return ycomp[newpos]

    global LAST_NC, LAST_RUN_S
    LAST_NC = nc
    import time as _time

    _t0 = _time.time()
    res = bass_utils.run_bass_kernel_spmd(
        nc,
        in_maps,
        core_ids=list(range(C)),
        trace=cfg.get("TRACE", False),
    )
    LAST_RUN_S = _time.time() - _t0
    LAST_RESULTS = res

    ycomp = np.empty((N, OUT), np.float32)
    for k in range(C):
        ycomp[k * NPC : (k + 1) * NPC] = res.results[k]["ypadT"][:, :NPC].T
    return ycomp[newpos]


# revision 22
# speedup vs baseline: 3.0226x; 1.0618x over previous
"""GCN block (GraphConv + BatchNorm1d + ReLU) on 8 Trainium2 NeuronCores.

Strategy: partition dst nodes across the 8 cores; every core keeps the FULL
x table (an external input, so it is staged for free) in its HBM and gathers
x[src] rows directly — the weight is applied AFTER aggregation, which is
exact because aggregation is linear:

    agg[d] = sum_e  nsrc[src_e] * ndst[d] * x[src_e]        (segmented sum)
    y_pre[d] = agg[d] @ W + b ; h = relu(y_pre) ; BN(h)

This removes the h AllGather of the original design entirely (it cost
~700us of collective time on the critical path).

Layout is feature-major on chip ([feature, node]) so bias and the BN affine
are per-partition ACT ops, and BN batch sums fall out of the ACT
accumulator for free.

The dst->core assignment is ours to choose, so nodes are assigned to
(core, group) slots by a degree-profile-aware round-robin (nodes with equal
per-bank in-degree profiles are dealt cyclically across all 784 slots).
This equalizes every (group, bank) bucket's edge count across the 8 cores,
collapsing the shared-NEFF padding slack from ~11% to ~2%; buckets are then
padded to 16-slot granularity (the dma_gather index wrap).  128-edge blocks
may straddle group boundaries: each (block, group) pair in the shared
schedule gets its own one-hot matrix M built from a per-pair pre-shifted
dst-offset column (values outside [0,128) give zero columns, so foreign
and pad slots contribute exactly 0).  Per-edge degree norms are folded into
M by the dual-op tensor_scalar (is_equal then mult) at zero extra cost.

Groups are processed in chunks of GC=3: each group owns a dedicated
PSUM-bank pair whose accumulation chain spans all 4 src banks of its chunk
(6 seg banks + 2 W banks = all 8).  On group completion: agg pair -> SBUF
bf16, W matmul, relu(+bias ptr) with ACT accumulator emitting BN sums,
Square pass emitting sum-of-squares; AllReduce [128,2]; per-partition
affine; y^T written to HBM (host permutes rows back).

Host-side work is limited to integer index bookkeeping and layout/dtype
transforms. All floating-point math runs on device.
"""
import math
import os
import sys

sys.path.insert(0, "/opt/trn_rl_repo")

import numpy as np

import concourse.bacc as bacc
import concourse.bass as bass
import concourse.mybir as mybir
import concourse.tile as tile
from concourse import bass_utils

F32 = mybir.dt.float32
BF16 = mybir.dt.bfloat16
I16 = mybir.dt.int16

CFG = dict(
    N=100000,
    E=1600000,
    IN=256,
    OUT=128,
    NCORES=8,
    GRP=128,          # dst nodes per segment group (= one-hot free dim)
    NBANKS=4,         # src banks (bank rows must stay < 32768 for int16 idx)
    XB=25000,         # rows per x bank
    GC=3,             # groups per chunk (2*GC psum seg banks + 2 W banks <= 8)
    BATCH_BLOCKS=48,  # gather batch cap in 128-edge blocks
    EPS=1e-5,
    TRACE=False,
)

LAST_RESULTS = None  # set by kernel() for test harness introspection
LAST_NC = None
LAST_RUN_S = None


def _ceil_div(a, b):
    return (a + b - 1) // b


def _wrap16(idx, ncols):
    """int16 idx list -> [128, ncols] tile: idx i at [i%16, i//16], replicated
    8x across the 16-partition groups (one copy per GpSimd Q7 core)."""
    n = idx.shape[0]
    assert n == ncols * 16
    w = np.ascontiguousarray(idx.reshape(ncols, 16).T)
    return np.tile(w, (8, 1))


def _balance_nodes(cfg, src, dst):
    """Assign dst nodes to (core, group) slots so that every (group-pos,
    bank) bucket has a near-equal edge count on all 8 cores.  Nodes with
    identical per-bank in-degree profiles are dealt round-robin across all
    slots.  Returns newpos[node] (position in the concatenated core
    layout)."""
    N = cfg["N"]
    C, NG, GRP, XB = cfg["NCORES"], _ceil_div(N // cfg["NCORES"], cfg["GRP"]), cfg["GRP"], cfg["XB"]
    NPC = N // C
    NS = C * NG
    last_w = NPC - (NG - 1) * GRP

    bank_e = src // XB
    prof = np.bincount(dst * 4 + bank_e, minlength=N * 4).reshape(N, 4)
    _, inv = np.unique(prof, axis=0, return_inverse=True)
    order_nodes = np.argsort(inv, kind="stable")
    cls_sorted = inv[order_nodes]
    seg = np.flatnonzero(np.diff(cls_sorted)) + 1
    seg_starts = np.concatenate([[0], seg, [N]])

    slot_of = np.empty(N, np.int64)
    ptr = 0
    for i in range(len(seg_starts) - 1):
        a, b = seg_starts[i], seg_starts[i + 1]
        m = b - a
        slot_of[order_nodes[a:b]] = (np.arange(m) + ptr) % NS
        ptr = (ptr + m) % NS
    # capacity fix: slots (k, NG-1) hold only last_w nodes
    cap = np.full(NS, GRP, np.int64)
    cap[(NG - 1) * C :] = last_w  # slot id s: gp = s // C, core = s % C
    by_slot = np.argsort(slot_of, kind="stable")
    fill = np.bincount(slot_of, minlength=NS)
    cum = np.concatenate([[0], np.cumsum(fill)])
    moved = []
    for s in range(NS):
        if fill[s] > cap[s]:
            moved.extend(by_slot[cum[s] + cap[s] : cum[s + 1]])
    if moved:
        room_slots = np.repeat(
            np.arange(NS), np.maximum(cap - fill, 0)
        )[: len(moved)]
        slot_of[np.array(moved)] = room_slots
    # final positions
    by_slot = np.argsort(slot_of, kind="stable")
    fill = np.bincount(slot_of, minlength=NS)
    assert (fill == cap).all()
    offs = np.arange(N) - np.repeat(
        np.concatenate([[0], np.cumsum(fill)])[:-1], fill
    )
    s_sorted = slot_of[by_slot]
    newpos = np.empty(N, np.int64)
    newpos[by_slot] = (s_sorted % C) * NPC + (s_sorted // C) * GRP + offs
    return newpos


def _preprocess(cfg, src, dst):
    """Bucket edges by (owner core, chunk, src bank, group) under the
    balanced node assignment; build the shared (block, group) pair schedule
    and per-core gather-index / dst-offset / per-edge-degree arrays."""
    N, E = cfg["N"], cfg["E"]
    C, NBANKS, GRP, GC = cfg["NCORES"], cfg["NBANKS"], cfg["GRP"], cfg["GC"]
    XB = cfg["XB"]
    NPC = N // C
    NG = _ceil_div(NPC, GRP)
    NCH = _ceil_div(NG, GC)
    assert XB * NBANKS == N and XB < 32768

    src = src.astype(np.int64)
    dst = dst.astype(np.int64)
    deg_out = np.bincount(src, minlength=N).astype(np.float32)
    deg_in = np.bincount(dst, minlength=N).astype(np.float32)

    newpos = _balance_nodes(cfg, src, dst)
    dstN = newpos[dst]

    owner = dstN // NPC
    g_of = (dstN % NPC) // GRP
    ch_of = g_of // GC
    gi_of = g_of - ch_of * GC
    bank = src // XB
    key = ((owner * NCH + ch_of) * NBANKS + bank) * GC + gi_of
    order = np.argsort(key, kind="stable")
    s_src = src[order]
    s_dstN = dstN[order]
    s_dstO = dst[order]
    s_key = key[order]

    nkey = C * NCH * NBANKS * GC
    counts = np.bincount(key, minlength=nkey).reshape(C, NCH, NBANKS, GC)
    cmax = counts.max(axis=0)  # [NCH, NBANKS, GC]
    R = ((cmax + 15) // 16) * 16
    for ch in range(NCH):
        ngr = min(GC, NG - ch * GC)
        R[ch, 0, :ngr] = np.maximum(R[ch, 0, :ngr], 16)  # bank-0 run exists
        R[ch, :, ngr:] = 0

    # stream layout: per (chunk, bank): runs at 16-slot granularity, stream
    # rounded up to whole 128-slot blocks
    run_off = {}
    stream_blk0 = {}
    pos = 0
    for ch in range(NCH):
        ngr = min(GC, NG - ch * GC)
        for b in range(NBANKS):
            assert pos % 128 == 0
            stream_blk0[(ch, b)] = pos // 128
            for gi in range(ngr):
                if R[ch, b, gi] > 0:
                    run_off[(ch, b, gi)] = pos
                    pos += int(R[ch, b, gi])
            pos = _ceil_div(pos, 128) * 128
    nidx_tot = pos
    nb_tot = nidx_tot // 128

    # (block, group) pair schedule + per-group chain flags
    pairs = []           # (t, g)
    group_pairs = {}     # g -> [pair indices in emission order]
    block_pairs = [[] for _ in range(nb_tot)]
    for ch in range(NCH):
        ngr = min(GC, NG - ch * GC)
        for b in range(NBANKS):
            for gi in range(ngr):
                if R[ch, b, gi] == 0:
                    continue
                g = ch * GC + gi
                r0 = run_off[(ch, b, gi)]
                r1 = r0 + int(R[ch, b, gi])
                for t in range(r0 // 128, (r1 - 1) // 128 + 1):
                    p = len(pairs)
                    pairs.append((t, g))
                    group_pairs.setdefault(g, []).append(p)
                    block_pairs[t].append(p)
    npairs = len(pairs)
    # valid slot count per stream tail block (gather is trimmed to the
    # 16-granular used length; matmuls must not read unwritten Gt rows)
    used_end_of = {}
    for ch in range(NCH):
        ngr = min(GC, NG - ch * GC)
        for b in range(NBANKS):
            ue = max(
                (run_off[(ch, b, gi)] + int(R[ch, b, gi]))
                for gi in range(ngr)
                if R[ch, b, gi] > 0
            )
            used_end_of[(ch, b)] = ue
    blk_kk = np.full(nb_tot, 128, np.int64)
    for (ch, b), ue in used_end_of.items():
        t = (ue - 1) // 128
        if ue - t * 128 < 128:
            blk_kk[t] = ue - t * 128
    pair_info = []
    starts_set = {gp[0] for gp in group_pairs.values()}
    stops_set = {gp[-1] for gp in group_pairs.values()}
    for p, (t, g) in enumerate(pairs):
        pair_info.append((g, p in starts_set, p in stops_set, int(blk_kk[t])))
    # dedupe: a straddling run can emit two pairs (t, g) for consecutive
    # runs of the same g in different banks mapping to the same t — they
    # are distinct pairs (per-bank), which is fine for the psum chain.

    # gather batches: consecutive blocks within one (chunk, bank) stream.
    # nidx (16-granular) trims the stream-tail pad slots from the gather:
    # those slots have no (block, group) pairs, so they are never read.
    batches = []  # (bank, first_block, n_blocks, n_gather_idxs)
    for ch in range(NCH):
        ngr = min(GC, NG - ch * GC)
        for b in range(NBANKS):
            t0 = stream_blk0[(ch, b)]
            t1 = stream_blk0.get((ch, b + 1))
            if t1 is None:
                t1 = stream_blk0.get((ch + 1, 0), nb_tot)
            used_end = max(
                run_off[(ch, b, gi)] + int(R[ch, b, gi])
                for gi in range(ngr)
                if R[ch, b, gi] > 0
            )
            rem = t1 - t0
            t = t0
            while rem > 0:
                nb = min(rem, cfg["BATCH_BLOCKS"])
                nidx = min(nb * 128, max(used_end - t * 128, 0))
                if nidx > 0:
                    batches.append((b, t, nb, nidx))
                t += nb
                rem -= nb

    # per (k, ch, b, gi) boundaries in the sorted edge stream
    bkeys = np.arange(nkey)
    bstarts = np.searchsorted(s_key, bkeys).reshape(C, NCH, NBANKS, GC)
    bends = np.searchsorted(s_key, bkeys, side="right").reshape(C, NCH, NBANKS, GC)

    gidx_cores = []
    doff_cores = []
    dgo_cores = []
    dgi_cores = []
    for k in range(C):
        gidx = np.zeros(nidx_tot, np.int16)
        dmod = np.full(nidx_tot, -1.0e6, np.float32)
        dgo = np.ones(nidx_tot, np.float32)
        dgi = np.ones(nidx_tot, np.float32)
        for (ch, b, gi), p0 in run_off.items():
            s, e = int(bstarts[k, ch, b, gi]), int(bends[k, ch, b, gi])
            cnt = e - s
            if cnt == 0:
                continue
            gidx[p0 : p0 + cnt] = (s_src[s:e] % XB).astype(np.int16)
            dmod[p0 : p0 + cnt] = (s_dstN[s:e] % NPC).astype(np.float32)
            dgo[p0 : p0 + cnt] = deg_out[s_src[s:e]]
            dgi[p0 : p0 + cnt] = deg_in[s_dstO[s:e]]
        dmod2 = dmod.reshape(nb_tot, 128)
        doff = np.empty((npairs, 128), np.float32)
        for p, (t, g) in enumerate(pairs):
            doff[p] = dmod2[t] - np.float32(g * GRP)
        gidx_cores.append(_wrap16(gidx, nidx_tot // 16))
        doff_cores.append(np.ascontiguousarray(doff.T))
        dgo_cores.append(np.ascontiguousarray(dgo.reshape(nb_tot, 128).T))
        dgi_cores.append(np.ascontiguousarray(dgi.reshape(nb_tot, 128).T))

    meta = dict(
        NPC=NPC,
        NG=NG,
        nidx_tot=nidx_tot,
        nb_tot=nb_tot,
        npairs=npairs,
        pairs=pairs,
        pair_info=pair_info,
        block_pairs=block_pairs,
        batches=batches,
        newpos=newpos,
    )
    return meta, gidx_cores, doff_cores, dgo_cores, dgi_cores


def _build_nc(cfg, meta):
    N, IN, OUT, C = cfg["N"], cfg["IN"], cfg["OUT"], cfg["NCORES"]
    GRP, NBANKS, XB, GC = cfg["GRP"], cfg["NBANKS"], cfg["XB"], cfg["GC"]
    NPC, NG = meta["NPC"], meta["NG"]
    nidx_tot, nb_tot = meta["nidx_tot"], meta["nb_tot"]
    npairs = meta["npairs"]
    pair_info = meta["pair_info"]
    block_pairs = meta["block_pairs"]
    batches = meta["batches"]
    XK = _ceil_div(IN, 128)
    assert OUT == 128 and GRP == 128 and IN == 256
    last_w = NPC - (NG - 1) * GRP  # valid dst cols in the last group

    nc = bacc.Bacc(
        "TRN2", target_bir_lowering=False, debug=False, num_devices=C
    )

    # ---- external inputs ----
    xb = [
        nc.dram_tensor(f"xb{q}", [XB, IN], BF16, kind="ExternalInput")
        for q in range(NBANKS)
    ]
    wt = [
        nc.dram_tensor(f"wt{j}", [128, OUT], BF16, kind="ExternalInput")
        for j in range(XK)
    ]
    gidx_d = nc.dram_tensor("gidx", [128, nidx_tot // 16], I16, kind="ExternalInput")
    doff_d = nc.dram_tensor("doff", [128, npairs], F32, kind="ExternalInput")
    dgo_d = nc.dram_tensor("dgo", [128, nb_tot], F32, kind="ExternalInput")
    dgi_d = nc.dram_tensor("dgi", [128, nb_tot], F32, kind="ExternalInput")
    iota_d = nc.dram_tensor("iota", [128, GRP], BF16, kind="ExternalInput")
    btc_d = nc.dram_tensor("btc", [OUT, 1], F32, kind="ExternalInput")
    gmc_d = nc.dram_tensor("gmc", [OUT, 1], F32, kind="ExternalInput")
    bbc_d = nc.dram_tensor("bbc", [OUT, 1], F32, kind="ExternalInput")

    ypadT_d = nc.dram_tensor("ypadT", [OUT, NG * GRP], BF16, kind="ExternalOutput")

    bmax = max(nb for _, _, nb, _ in batches)

    with tile.TileContext(nc) as tc:
        with (
            tc.tile_pool(name="const", bufs=1) as cpool,
            tc.tile_pool(name="dram", bufs=1, space="DRAM") as dpool,
            tc.tile_pool(name="agg", bufs=1) as apool,
            tc.tile_pool(name="gath", bufs=3) as gpool,
            tc.tile_pool(name="mpool", bufs=6) as mpool,
            tc.tile_pool(name="asb", bufs=2) as asbp,
            tc.tile_pool(name="sq", bufs=2) as sqp,
            tc.tile_pool(name="pseg", bufs=1, space="PSUM") as psegp,
            tc.tile_pool(name="pw", bufs=2, space="PSUM") as pwp,
        ):
            # ---- constants / small tiles ----
            iota_t = cpool.tile([128, GRP], BF16)
            btc_t = cpool.tile([OUT, 1], F32)
            gmc_t = cpool.tile([OUT, 1], F32)
            bbc_t = cpool.tile([OUT, 1], F32)
            gidx_t = cpool.tile([128, nidx_tot // 16], I16)
            doff_t = cpool.tile([128, npairs], F32)
            dgo_t = cpool.tile([128, nb_tot], F32)
            dgi_t = cpool.tile([128, nb_tot], F32)
            stats_s = cpool.tile([OUT, NG], F32)
            stats_q = cpool.tile([OUT, NG], F32)
            wts = []
            for j in range(XK):
                wtile = cpool.tile([128, OUT], BF16, name=f"wt_s{j}")
                nc.sync.dma_start(wtile[:], wt[j][:])
                wts.append(wtile)

            # split the big index load so the first gather batches and the
            # first M-gens don't wait for the whole-tensor DMAs
            gcols = nidx_tot // 16
            gsplits = [0, min(256, gcols), min(2048, gcols), gcols]
            for a, z in zip(gsplits, gsplits[1:]):
                if z > a:
                    nc.sync.dma_start(gidx_t[:, a:z], gidx_d[:, a:z])
            nc.sync.dma_start(iota_t[:], iota_d[:])
            bsplit = [0, min(256, nb_tot), nb_tot]
            for a, z in zip(bsplit, bsplit[1:]):
                if z > a:
                    nc.sync.dma_start(dgo_t[:, a:z], dgo_d[:, a:z])
                    nc.sync.dma_start(dgi_t[:, a:z], dgi_d[:, a:z])
            psplit = [0, min(256, npairs), npairs]
            for a, z in zip(psplit, psplit[1:]):
                if z > a:
                    nc.sync.dma_start(doff_t[:, a:z], doff_d[:, a:z])
            nc.sync.dma_start(btc_t[:], btc_d[:])
            nc.sync.dma_start(gmc_t[:], gmc_d[:])
            nc.sync.dma_start(bbc_t[:], bbc_d[:])

            # per-edge norm scale s = rsqrt(max(dgo,1)) * rsqrt(max(dgi,1))
            # (computed in-place in dgo_t; dgi_t is scratch after this),
            # in two column segments so early blocks unblock fast
            s_t = dgo_t
            for a, z in zip(bsplit, bsplit[1:]):
                if z <= a:
                    continue
                for deg_t in (dgo_t, dgi_t):
                    nc.vector.tensor_scalar(
                        deg_t[:, a:z], deg_t[:, a:z], 1.0, None,
                        op0=mybir.AluOpType.max,
                    )
                    nc.vector.reciprocal(deg_t[:, a:z], deg_t[:, a:z])
                    nc.scalar.activation(
                        deg_t[:, a:z], deg_t[:, a:z],
                        mybir.ActivationFunctionType.Sqrt,
                    )
                nc.vector.tensor_mul(s_t[:, a:z], dgo_t[:, a:z], dgi_t[:, a:z])

            # h table (feature-major, bf16): agg_t[:, g, d] = h[o, g*128+d]
            agg_t = apool.tile([OUT, NG, GRP], BF16)
            # zero the last group's pad columns (stats square-pass reads them)
            nc.gpsimd.memset(agg_t[:, NG - 1, :], 0.0)

            # internal DRAM for the BN-stats collective (AllGather is ~2x
            # cheaper than AllReduce in fixed cost; reduce locally instead)
            stats_in = dpool.tile([OUT, 2], F32)
            stats_out = dpool.tile([C * OUT, 2], F32, addr_space="Shared")

            # ---- main loop: gather + one-hot matmul segmented sum ----
            cur_ps = {}  # gi -> (psA, psB)
            for bank, t0, nblk, nidx in batches:
                Gt = gpool.tile([128, bmax, IN], BF16, tag="G")
                nc.gpsimd.dma_gather(
                    Gt[:, : _ceil_div(nidx, 128), :],
                    xb[bank][:],
                    gidx_t[:, t0 * 8 : t0 * 8 + nidx // 16],
                    nidx,
                    nidx,
                    IN,
                    single_packet=False,
                )
                for j in range(nblk):
                    t = t0 + j
                    for p in block_pairs[t]:
                        g, is_start, is_stop, kk = pair_info[p]
                        gi = g % GC
                        Mt = mpool.tile([128, GRP], BF16, tag="M")
                        nc.vector.tensor_scalar(
                            Mt[:],
                            iota_t[:],
                            doff_t[:, p : p + 1],
                            s_t[:, t : t + 1],
                            op0=mybir.AluOpType.is_equal,
                            op1=mybir.AluOpType.mult,
                        )
                        if is_start:
                            psA = psegp.tile(
                                [128, GRP], F32, tag=f"sA{gi}", name=f"psA{gi}"
                            )
                            psB = psegp.tile(
                                [128, GRP], F32, tag=f"sB{gi}", name=f"psB{gi}"
                            )
                            cur_ps[gi] = (psA, psB)
                        psA, psB = cur_ps[gi]
                        nc.tensor.matmul(
                            psA[:], Gt[:kk, j, 0:128], Mt[:kk, :],
                            start=is_start, stop=is_stop,
                        )
                        nc.tensor.matmul(
                            psB[:], Gt[:kk, j, 128:256], Mt[:kk, :],
                            start=is_start, stop=is_stop,
                        )
                        if not is_stop:
                            continue
                        aggA = asbp.tile([128, GRP], BF16, tag="aggA")
                        aggB = asbp.tile([128, GRP], BF16, tag="aggB")
                        nc.scalar.activation(
                            aggA[:], psA[:], mybir.ActivationFunctionType.Copy
                        )
                        nc.scalar.activation(
                            aggB[:], psB[:], mybir.ActivationFunctionType.Copy
                        )
                        pso = pwp.tile([OUT, GRP], F32, tag="w")
                        nc.tensor.matmul(
                            pso[:], wts[0][:], aggA[:], start=True, stop=False
                        )
                        nc.tensor.matmul(
                            pso[:], wts[1][:], aggB[:], start=False, stop=True
                        )
                        w = GRP if g < NG - 1 else last_w
                        nc.scalar.activation(
                            agg_t[:, g, :w],
                            pso[:, :w],
                            mybir.ActivationFunctionType.Relu,
                            bias=btc_t[:, 0:1],
                            accum_out=stats_s[:, g : g + 1],
                        )
                        sqt = sqp.tile([OUT, GRP], BF16, tag="sq")
                        nc.scalar.activation(
                            sqt[:, :w],
                            agg_t[:, g, :w],
                            mybir.ActivationFunctionType.Square,
                            accum_out=stats_q[:, g : g + 1],
                        )

            # ---- BN stats AllReduce + affine finalize ----
            stsb = cpool.tile([OUT, 2], F32)
            nc.vector.tensor_reduce(
                stsb[:, 0:1], stats_s[:], mybir.AxisListType.X, mybir.AluOpType.add
            )
            nc.vector.tensor_reduce(
                stsb[:, 1:2], stats_q[:], mybir.AxisListType.X, mybir.AluOpType.add
            )
            nc.sync.dma_start(stats_in[:], stsb[:])
            nc.gpsimd.collective_compute(
                "AllGather",
                mybir.AluOpType.bypass,
                replica_groups=[list(range(C))],
                ins=[stats_in[:]],
                outs=[stats_out[:]],
            )
            stall = cpool.tile([OUT, C, 2], F32)
            for c in range(C):
                nc.sync.dma_start(
                    stall[:, c, :], stats_out[c * OUT : (c + 1) * OUT, :]
                )
            strb = cpool.tile([OUT, 2], F32)
            nc.vector.tensor_reduce(
                strb[:],
                stall[:].rearrange("p c f -> p f c"),
                mybir.AxisListType.X,
                mybir.AluOpType.add,
            )

            mu = cpool.tile([OUT, 1], F32)
            ex2 = cpool.tile([OUT, 1], F32)
            var = cpool.tile([OUT, 1], F32)
            S_t = cpool.tile([OUT, 1], F32)
            T_t = cpool.tile([OUT, 1], F32)
            inv_n = 1.0 / float(N)
            nc.scalar.activation(
                mu[:], strb[:, 0:1], mybir.ActivationFunctionType.Copy, scale=inv_n
            )
            nc.scalar.activation(
                ex2[:], strb[:, 1:2], mybir.ActivationFunctionType.Copy, scale=inv_n
            )
            nc.scalar.activation(var[:], mu[:], mybir.ActivationFunctionType.Square)
            nc.vector.tensor_sub(var[:], ex2[:], var[:])
            # var <- rsqrt(var + eps) (ACT Rsqrt is banned for accuracy)
            nc.scalar.activation(
                var[:], var[:], mybir.ActivationFunctionType.Copy,
                bias=float(cfg["EPS"]),
            )
            nc.vector.reciprocal(var[:], var[:])
            nc.scalar.activation(var[:], var[:], mybir.ActivationFunctionType.Sqrt)
            nc.vector.tensor_mul(S_t[:], gmc_t[:], var[:])
            nc.vector.tensor_mul(T_t[:], mu[:], S_t[:])
            nc.vector.tensor_sub(T_t[:], bbc_t[:], T_t[:])

            # ---- apply affine: y = h*S + T (ACT/DVE quarters, DMA
            # interleaved so the writeback overlaps the affine) ----
            ypadT_view = ypadT_d[:].rearrange("p (g f) -> p g f", g=NG)
            qs = [0, NG // 4, NG // 2, 3 * NG // 4, NG]
            for qi in range(4):
                a, z = qs[qi], qs[qi + 1]
                if qi % 2 == 0:
                    nc.scalar.activation(
                        agg_t[:, a:z, :],
                        agg_t[:, a:z, :],
                        mybir.ActivationFunctionType.Identity,
                        bias=T_t[:, 0:1],
                        scale=S_t[:, 0:1],
                    )
                else:
                    nc.vector.tensor_scalar(
                        agg_t[:, a:z, :],
                        agg_t[:, a:z, :],
                        S_t[:, 0:1],
                        T_t[:, 0:1],
                        op0=mybir.AluOpType.mult,
                        op1=mybir.AluOpType.add,
                    )
                nc.sync.dma_start(ypadT_view[:, a:z, :], agg_t[:, a:z, :])

    nc.compile()
    return nc


def kernel(x, src, dst, W, b, gamma, beta):
    global LAST_RESULTS
    cfg = CFG
    N, E, IN, OUT, C = cfg["N"], cfg["E"], cfg["IN"], cfg["OUT"], cfg["NCORES"]
    GRP, XB, NBANKS = cfg["GRP"], cfg["XB"], cfg["NBANKS"]
    assert x.shape == (N, IN) and W.shape == (IN, OUT)
    assert src.shape == (E,) and dst.shape == (E,)

    meta, gidx_cores, doff_cores, dgo_cores, dgi_cores = _preprocess(cfg, src, dst)
    NPC, NG = meta["NPC"], meta["NG"]
    newpos = meta["newpos"]
    XK = _ceil_div(IN, 128)

    nc = _build_nc(cfg, meta)

    import ml_dtypes

    xbf = np.asarray(x, np.float32).astype(ml_dtypes.bfloat16)  # [N, IN]
    Wn = np.asarray(W, np.float32)

    iota = np.tile(
        np.arange(GRP, dtype=np.float32)[None, :], (128, 1)
    ).astype(ml_dtypes.bfloat16)
    btc = np.ascontiguousarray(np.asarray(b, np.float32)[:, None])
    gmc = np.ascontiguousarray(np.asarray(gamma, np.float32)[:, None])
    bbc = np.ascontiguousarray(np.asarray(beta, np.float32)[:, None])

    xbanks = {
        f"xb{q}": np.ascontiguousarray(xbf[q * XB : (q + 1) * XB, :])
        for q in range(NBANKS)
    }
    wmap = {
        f"wt{j}": np.ascontiguousarray(
            Wn[j * 128 : (j + 1) * 128, :]
        ).astype(ml_dtypes.bfloat16)
        for j in range(XK)
    }

    in_maps = []
    for k in range(C):
        im = {
            "gidx": gidx_cores[k],
            "doff": doff_cores[k],
            "dgo": dgo_cores[k],
            "dgi": dgi_cores[k],
            "iota": iota,
            "btc": btc,
            "gmc": gmc,
            "bbc": bbc,
        }
        im.update(xbanks)
        im.update(wmap)
        in_maps.append(im)

    if cfg.get("SIM"):
        from concourse.bass_interp import MultiCoreSim

        sim = MultiCoreSim(nc, num_cores=C)
        for k, core_sim in sim.cores.items():
            for name, val in in_maps[k].items():
                core_sim.tensor(name)[:] = val
        sim.simulate()
        ycomp = np.empty((N, OUT), np.float32)
        for k in range(C):
            ycomp[k * NPC : (k + 1) * NPC] = (
                sim.cores[k].tensor("ypadT")[:, :NPC].astype(np.float32).T
            )
        return ycomp[newpos]

    global LAST_NC, LAST_RUN_S
    LAST_NC = nc
    import time as _time

    _t0 = _time.time()
    res = bass_utils.run_bass_kernel_spmd(
        nc,
        in_maps,
        core_ids=list(range(C)),
        trace=cfg.get("TRACE", False),
    )
    LAST_RUN_S = _time.time() - _t0
    LAST_RESULTS = res

    ycomp = np.empty((N, OUT), np.float32)
    for k in range(C):
        ycomp[k * NPC : (k + 1) * NPC] = (
            res.results[k]["ypadT"][:, :NPC].astype(np.float32).T
        )
    return ycomp[newpos]


# revision 29
# speedup vs baseline: 3.5218x; 1.1652x over previous
"""GCN block (GraphConv + BatchNorm1d + ReLU) on 8 Trainium2 NeuronCores.

Strategy: partition dst nodes across the 8 cores; every core keeps the FULL
x table (an external input, so it is staged for free) in its HBM and gathers
x[src] rows directly — the weight is applied AFTER aggregation, which is
exact because aggregation is linear:

    agg[d] = sum_e  nsrc[src_e] * ndst[d] * x[src_e]        (segmented sum)
    y_pre[d] = agg[d] @ W + b ; h = relu(y_pre) ; BN(h)

This removes the h AllGather of the original design entirely (it cost
~700us of collective time on the critical path).

Layout is feature-major on chip ([feature, node]) so bias and the BN affine
are per-partition ACT ops, and BN batch sums fall out of the ACT
accumulator for free.

The dst->core assignment is ours to choose, so nodes are assigned to
(core, group) slots by a degree-profile-aware round-robin (nodes with equal
per-bank in-degree profiles are dealt cyclically across all 784 slots).
This equalizes every (group, bank) bucket's edge count across the 8 cores,
collapsing the shared-NEFF padding slack from ~11% to ~2%; buckets are then
padded to 16-slot granularity (the dma_gather index wrap).  128-edge blocks
may straddle group boundaries: each (block, group) pair in the shared
schedule gets its own one-hot matrix M built from a per-pair pre-shifted
dst-offset column (values outside [0,128) give zero columns, so foreign
and pad slots contribute exactly 0).  Per-edge degree norms are folded into
M by the dual-op tensor_scalar (is_equal then mult) at zero extra cost.

Groups are processed in chunks of GC=3: each group owns a dedicated
PSUM-bank pair whose accumulation chain spans all 4 src banks of its chunk
(6 seg banks + 2 W banks = all 8).  On group completion: agg pair -> SBUF
bf16, W matmul, relu(+bias ptr) with ACT accumulator emitting BN sums,
Square pass emitting sum-of-squares; AllReduce [128,2]; per-partition
affine; y^T written to HBM (host permutes rows back).

Host-side work is limited to integer index bookkeeping and layout/dtype
transforms. All floating-point math runs on device.
"""
import math
import os
import sys

sys.path.insert(0, "/opt/trn_rl_repo")

import numpy as np

import concourse.bacc as bacc
import concourse.bass as bass
import concourse.mybir as mybir
import concourse.tile as tile
from concourse import bass_utils

F32 = mybir.dt.float32
BF16 = mybir.dt.bfloat16
I16 = mybir.dt.int16

CFG = dict(
    N=100000,
    E=1600000,
    IN=256,
    OUT=128,
    NCORES=8,
    GRP=128,          # dst nodes per segment group (= one-hot free dim)
    NBANKS=4,         # src banks (bank rows must stay < 32768 for int16 idx)
    XB=25000,         # rows per x bank
    GC=3,             # groups per chunk (2*GC psum seg banks + 2 W banks <= 8)
    BATCH_BLOCKS=48,  # gather batch cap in 128-edge blocks
    EPS=1e-5,
    TRACE=False,
)

LAST_RESULTS = None  # set by kernel() for test harness introspection
LAST_NC = None
LAST_RUN_S = None


def _ceil_div(a, b):
    return (a + b - 1) // b


def _wrap16(idx, ncols):
    """int16 idx list -> [128, ncols] tile: idx i at [i%16, i//16], replicated
    8x across the 16-partition groups (one copy per GpSimd Q7 core)."""
    n = idx.shape[0]
    assert n == ncols * 16
    w = np.ascontiguousarray(idx.reshape(ncols, 16).T)
    return np.tile(w, (8, 1))


def _balance_nodes(cfg, src, dst):
    """Assign dst nodes to (core, group) slots so that every (group-pos,
    bank) bucket has a near-equal edge count on all 8 cores.  Nodes with
    identical per-bank in-degree profiles are dealt round-robin across all
    slots.  Returns newpos[node] (position in the concatenated core
    layout)."""
    N = cfg["N"]
    C, NG, GRP, XB = cfg["NCORES"], _ceil_div(N // cfg["NCORES"], cfg["GRP"]), cfg["GRP"], cfg["XB"]
    NPC = N // C
    NS = C * NG
    last_w = NPC - (NG - 1) * GRP

    bank_e = src // XB
    prof = np.bincount(dst * 4 + bank_e, minlength=N * 4).reshape(N, 4)
    _, inv = np.unique(prof, axis=0, return_inverse=True)
    order_nodes = np.argsort(inv, kind="stable")
    cls_sorted = inv[order_nodes]
    seg = np.flatnonzero(np.diff(cls_sorted)) + 1
    seg_starts = np.concatenate([[0], seg, [N]])

    slot_of = np.empty(N, np.int64)
    ptr = 0
    for i in range(len(seg_starts) - 1):
        a, b = seg_starts[i], seg_starts[i + 1]
        m = b - a
        slot_of[order_nodes[a:b]] = (np.arange(m) + ptr) % NS
        ptr = (ptr + m) % NS
    # capacity fix: slots (k, NG-1) hold only last_w nodes
    cap = np.full(NS, GRP, np.int64)
    cap[(NG - 1) * C :] = last_w  # slot id s: gp = s // C, core = s % C
    by_slot = np.argsort(slot_of, kind="stable")
    fill = np.bincount(slot_of, minlength=NS)
    cum = np.concatenate([[0], np.cumsum(fill)])
    moved = []
    for s in range(NS):
        if fill[s] > cap[s]:
            moved.extend(by_slot[cum[s] + cap[s] : cum[s + 1]])
    if moved:
        room_slots = np.repeat(
            np.arange(NS), np.maximum(cap - fill, 0)
        )[: len(moved)]
        slot_of[np.array(moved)] = room_slots
    # final positions
    by_slot = np.argsort(slot_of, kind="stable")
    fill = np.bincount(slot_of, minlength=NS)
    assert (fill == cap).all()
    offs = np.arange(N) - np.repeat(
        np.concatenate([[0], np.cumsum(fill)])[:-1], fill
    )
    s_sorted = slot_of[by_slot]
    newpos = np.empty(N, np.int64)
    newpos[by_slot] = (s_sorted % C) * NPC + (s_sorted // C) * GRP + offs
    return newpos


def _preprocess(cfg, src, dst):
    """Bucket edges by (owner core, chunk, src bank, group) under the
    balanced node assignment; build the shared (block, group) pair schedule
    and per-core gather-index / dst-offset / per-edge-degree arrays."""
    N, E = cfg["N"], cfg["E"]
    C, NBANKS, GRP, GC = cfg["NCORES"], cfg["NBANKS"], cfg["GRP"], cfg["GC"]
    XB = cfg["XB"]
    NPC = N // C
    NG = _ceil_div(NPC, GRP)
    NCH = _ceil_div(NG, GC)
    assert XB * NBANKS == N and XB < 32768

    src = src.astype(np.int64)
    dst = dst.astype(np.int64)
    deg_out = np.bincount(src, minlength=N).astype(np.float32)
    deg_in = np.bincount(dst, minlength=N).astype(np.float32)

    newpos = _balance_nodes(cfg, src, dst)
    dstN = newpos[dst]

    owner = dstN // NPC
    g_of = (dstN % NPC) // GRP
    ch_of = g_of // GC
    gi_of = g_of - ch_of * GC
    bank = src // XB
    key = ((owner * NCH + ch_of) * NBANKS + bank) * GC + gi_of
    order = np.argsort(key, kind="stable")
    s_src = src[order]
    s_dstN = dstN[order]
    s_dstO = dst[order]
    s_key = key[order]

    nkey = C * NCH * NBANKS * GC
    counts = np.bincount(key, minlength=nkey).reshape(C, NCH, NBANKS, GC)
    cmax = counts.max(axis=0)  # [NCH, NBANKS, GC]
    R = ((cmax + 15) // 16) * 16
    for ch in range(NCH):
        ngr = min(GC, NG - ch * GC)
        R[ch, 0, :ngr] = np.maximum(R[ch, 0, :ngr], 16)  # bank-0 run exists
        R[ch, :, ngr:] = 0

    # stream layout: per (chunk, bank): runs at 16-slot granularity, stream
    # rounded up to whole 128-slot blocks
    run_off = {}
    stream_blk0 = {}
    pos = 0
    for ch in range(NCH):
        ngr = min(GC, NG - ch * GC)
        for b in range(NBANKS):
            assert pos % 128 == 0
            stream_blk0[(ch, b)] = pos // 128
            for gi in range(ngr):
                if R[ch, b, gi] > 0:
                    run_off[(ch, b, gi)] = pos
                    pos += int(R[ch, b, gi])
            pos = _ceil_div(pos, 128) * 128
    nidx_tot = pos
    nb_tot = nidx_tot // 128

    # (block, group) pair schedule + per-group chain flags
    pairs = []           # (t, g)
    group_pairs = {}     # g -> [pair indices in emission order]
    block_pairs = [[] for _ in range(nb_tot)]
    for ch in range(NCH):
        ngr = min(GC, NG - ch * GC)
        for b in range(NBANKS):
            for gi in range(ngr):
                if R[ch, b, gi] == 0:
                    continue
                g = ch * GC + gi
                r0 = run_off[(ch, b, gi)]
                r1 = r0 + int(R[ch, b, gi])
                for t in range(r0 // 128, (r1 - 1) // 128 + 1):
                    p = len(pairs)
                    pairs.append((t, g))
                    group_pairs.setdefault(g, []).append(p)
                    block_pairs[t].append(p)
    npairs = len(pairs)
    # valid slot count per stream tail block (gather is trimmed to the
    # 16-granular used length; matmuls must not read unwritten Gt rows)
    used_end_of = {}
    for ch in range(NCH):
        ngr = min(GC, NG - ch * GC)
        for b in range(NBANKS):
            ue = max(
                (run_off[(ch, b, gi)] + int(R[ch, b, gi]))
                for gi in range(ngr)
                if R[ch, b, gi] > 0
            )
            used_end_of[(ch, b)] = ue
    blk_kk = np.full(nb_tot, 128, np.int64)
    for (ch, b), ue in used_end_of.items():
        t = (ue - 1) // 128
        if ue - t * 128 < 128:
            blk_kk[t] = ue - t * 128
    pair_info = []
    starts_set = {gp[0] for gp in group_pairs.values()}
    stops_set = {gp[-1] for gp in group_pairs.values()}
    for p, (t, g) in enumerate(pairs):
        pair_info.append((g, p in starts_set, p in stops_set, int(blk_kk[t])))
    # dedupe: a straddling run can emit two pairs (t, g) for consecutive
    # runs of the same g in different banks mapping to the same t — they
    # are distinct pairs (per-bank), which is fine for the psum chain.

    # gather batches: consecutive blocks within one (chunk, bank) stream.
    # nidx (16-granular) trims the stream-tail pad slots from the gather:
    # those slots have no (block, group) pairs, so they are never read.
    batches = []  # (bank, first_block, n_blocks, n_gather_idxs)
    for ch in range(NCH):
        ngr = min(GC, NG - ch * GC)
        for b in range(NBANKS):
            t0 = stream_blk0[(ch, b)]
            t1 = stream_blk0.get((ch, b + 1))
            if t1 is None:
                t1 = stream_blk0.get((ch + 1, 0), nb_tot)
            used_end = max(
                run_off[(ch, b, gi)] + int(R[ch, b, gi])
                for gi in range(ngr)
                if R[ch, b, gi] > 0
            )
            rem = t1 - t0
            t = t0
            while rem > 0:
                nb = min(rem, cfg["BATCH_BLOCKS"])
                nidx = min(nb * 128, max(used_end - t * 128, 0))
                if nidx > 0:
                    batches.append((b, t, nb, nidx))
                t += nb
                rem -= nb
    # split the final batch so the tail dependency chain drains sooner
    if batches and batches[-1][2] >= 4:
        b, t, nb, nidx = batches.pop()
        nb1 = nb // 2
        n1 = min(nb1 * 128, nidx)
        batches.append((b, t, nb1, n1))
        if nidx > n1:
            batches.append((b, t + nb1, nb - nb1, nidx - n1))

    # per (k, ch, b, gi) boundaries in the sorted edge stream
    bkeys = np.arange(nkey)
    bstarts = np.searchsorted(s_key, bkeys).reshape(C, NCH, NBANKS, GC)
    bends = np.searchsorted(s_key, bkeys, side="right").reshape(C, NCH, NBANKS, GC)

    gidx_cores = []
    doff_cores = []
    dgo_cores = []
    dgi_cores = []
    for k in range(C):
        gidx = np.zeros(nidx_tot, np.int16)
        dmod = np.full(nidx_tot, -1.0e6, np.float32)
        dgo = np.ones(nidx_tot, np.float32)
        dgi = np.ones(nidx_tot, np.float32)
        for (ch, b, gi), p0 in run_off.items():
            s, e = int(bstarts[k, ch, b, gi]), int(bends[k, ch, b, gi])
            cnt = e - s
            if cnt == 0:
                continue
            gidx[p0 : p0 + cnt] = (s_src[s:e] % XB).astype(np.int16)
            dmod[p0 : p0 + cnt] = (s_dstN[s:e] % NPC).astype(np.float32)
            dgo[p0 : p0 + cnt] = deg_out[s_src[s:e]]
            dgi[p0 : p0 + cnt] = deg_in[s_dstO[s:e]]
        dmod2 = dmod.reshape(nb_tot, 128)
        doff = np.empty((npairs, 128), np.float32)
        for p, (t, g) in enumerate(pairs):
            doff[p] = dmod2[t] - np.float32(g * GRP)
        gidx_cores.append(_wrap16(gidx, nidx_tot // 16))
        doff_cores.append(np.ascontiguousarray(doff.T))
        dgo_cores.append(np.ascontiguousarray(dgo.reshape(nb_tot, 128).T))
        dgi_cores.append(np.ascontiguousarray(dgi.reshape(nb_tot, 128).T))

    meta = dict(
        NPC=NPC,
        NG=NG,
        nidx_tot=nidx_tot,
        nb_tot=nb_tot,
        npairs=npairs,
        pairs=pairs,
        pair_info=pair_info,
        block_pairs=block_pairs,
        batches=batches,
        newpos=newpos,
    )
    return meta, gidx_cores, doff_cores, dgo_cores, dgi_cores


def _build_nc(cfg, meta):
    N, IN, OUT, C = cfg["N"], cfg["IN"], cfg["OUT"], cfg["NCORES"]
    GRP, NBANKS, XB, GC = cfg["GRP"], cfg["NBANKS"], cfg["XB"], cfg["GC"]
    NPC, NG = meta["NPC"], meta["NG"]
    nidx_tot, nb_tot = meta["nidx_tot"], meta["nb_tot"]
    npairs = meta["npairs"]
    pair_info = meta["pair_info"]
    block_pairs = meta["block_pairs"]
    batches = meta["batches"]
    XK = _ceil_div(IN, 128)
    assert OUT == 128 and GRP == 128 and IN == 256
    last_w = NPC - (NG - 1) * GRP  # valid dst cols in the last group

    nc = bacc.Bacc(
        "TRN2", target_bir_lowering=False, debug=False, num_devices=C
    )

    # ---- external inputs ----
    xb = [
        nc.dram_tensor(f"xb{q}", [XB, IN], BF16, kind="ExternalInput")
        for q in range(NBANKS)
    ]
    wt = [
        nc.dram_tensor(f"wt{j}", [128, OUT], BF16, kind="ExternalInput")
        for j in range(XK)
    ]
    gidx_d = nc.dram_tensor("gidx", [128, nidx_tot // 16], I16, kind="ExternalInput")
    doff_d = nc.dram_tensor("doff", [128, npairs], F32, kind="ExternalInput")
    dgo_d = nc.dram_tensor("dgo", [128, nb_tot], BF16, kind="ExternalInput")
    dgi_d = nc.dram_tensor("dgi", [128, nb_tot], BF16, kind="ExternalInput")
    iota_d = nc.dram_tensor("iota", [128, GRP], BF16, kind="ExternalInput")
    btc_d = nc.dram_tensor("btc", [OUT, 1], F32, kind="ExternalInput")
    gmc_d = nc.dram_tensor("gmc", [OUT, 1], F32, kind="ExternalInput")
    bbc_d = nc.dram_tensor("bbc", [OUT, 1], F32, kind="ExternalInput")

    ypadT_d = nc.dram_tensor("ypadT", [OUT, NG * GRP], BF16, kind="ExternalOutput")

    bmax = max(nb for _, _, nb, _ in batches)

    with tile.TileContext(nc) as tc:
        with (
            tc.tile_pool(name="const", bufs=1) as cpool,
            tc.tile_pool(name="dram", bufs=1, space="DRAM") as dpool,
            tc.tile_pool(name="agg", bufs=1) as apool,
            tc.tile_pool(name="gath", bufs=4) as gpool,
            tc.tile_pool(name="mpool", bufs=8) as mpool,
            tc.tile_pool(name="asb", bufs=2) as asbp,
            tc.tile_pool(name="sq", bufs=2) as sqp,
            tc.tile_pool(name="pseg", bufs=1, space="PSUM") as psegp,
            tc.tile_pool(name="pw", bufs=2, space="PSUM") as pwp,
        ):
            # ---- constants / small tiles ----
            iota_t = cpool.tile([128, GRP], BF16)
            btc_t = cpool.tile([OUT, 1], F32)
            gmc_t = cpool.tile([OUT, 1], F32)
            bbc_t = cpool.tile([OUT, 1], F32)
            gidx_t = cpool.tile([128, nidx_tot // 16], I16)
            doff_t = cpool.tile([128, npairs], F32)
            dgo_t = cpool.tile([128, nb_tot], BF16)
            dgi_t = cpool.tile([128, nb_tot], BF16)
            s_t = cpool.tile([128, nb_tot], F32)
            r2_t = cpool.tile([128, nb_tot], F32)
            stats_s = cpool.tile([OUT, NG], F32)
            stats_q = cpool.tile([OUT, NG], F32)
            wts = []
            for j in range(XK):
                wtile = cpool.tile([128, OUT], BF16, name=f"wt_s{j}")
                nc.sync.dma_start(wtile[:], wt[j][:])
                wts.append(wtile)

            # split the big index load so the first gather batches and the
            # first M-gens don't wait for the whole-tensor DMAs
            gcols = nidx_tot // 16
            gsplits = [0, min(256, gcols), min(2048, gcols), gcols]
            for a, z in zip(gsplits, gsplits[1:]):
                if z > a:
                    nc.sync.dma_start(gidx_t[:, a:z], gidx_d[:, a:z])
            nc.sync.dma_start(iota_t[:], iota_d[:])
            bsplit = [0, min(256, nb_tot), nb_tot]
            for a, z in zip(bsplit, bsplit[1:]):
                if z > a:
                    nc.sync.dma_start(dgo_t[:, a:z], dgo_d[:, a:z])
                    nc.sync.dma_start(dgi_t[:, a:z], dgi_d[:, a:z])
            psplit = [0, min(256, npairs), npairs]
            for a, z in zip(psplit, psplit[1:]):
                if z > a:
                    nc.sync.dma_start(doff_t[:, a:z], doff_d[:, a:z])
            nc.sync.dma_start(btc_t[:], btc_d[:])
            nc.sync.dma_start(gmc_t[:], gmc_d[:])
            nc.sync.dma_start(bbc_t[:], bbc_d[:])

            # per-edge norm scale s = rsqrt(max(dgo,1)) * rsqrt(max(dgi,1))
            # in two column segments so early blocks unblock fast
            for a, z in zip(bsplit, bsplit[1:]):
                if z <= a:
                    continue
                for deg_t, out_t in ((dgo_t, s_t), (dgi_t, r2_t)):
                    nc.vector.tensor_scalar(
                        out_t[:, a:z], deg_t[:, a:z], 1.0, None,
                        op0=mybir.AluOpType.max,
                    )
                    nc.vector.reciprocal(out_t[:, a:z], out_t[:, a:z])
                    nc.scalar.activation(
                        out_t[:, a:z], out_t[:, a:z],
                        mybir.ActivationFunctionType.Sqrt,
                    )
                nc.vector.tensor_mul(s_t[:, a:z], s_t[:, a:z], r2_t[:, a:z])

            # h table (feature-major, bf16): agg_t[:, g, d] = h[o, g*128+d]
            agg_t = apool.tile([OUT, NG, GRP], BF16)
            # zero the last group's pad columns (stats square-pass reads them)
            nc.gpsimd.memset(agg_t[:, NG - 1, :], 0.0)

            # internal DRAM for the BN-stats collective (AllGather is ~2x
            # cheaper than AllReduce in fixed cost; reduce locally instead)
            stats_in = dpool.tile([OUT, 2], F32)
            stats_out = dpool.tile([C * OUT, 2], F32, addr_space="Shared")

            # ---- main loop: gather + one-hot matmul segmented sum ----
            cur_ps = {}  # gi -> (psA, psB)
            for bank, t0, nblk, nidx in batches:
                Gt = gpool.tile([128, bmax, IN], BF16, tag="G")
                nc.gpsimd.dma_gather(
                    Gt[:, : _ceil_div(nidx, 128), :],
                    xb[bank][:],
                    gidx_t[:, t0 * 8 : t0 * 8 + nidx // 16],
                    nidx,
                    nidx,
                    IN,
                    single_packet=False,
                )
                for j in range(nblk):
                    t = t0 + j
                    for p in block_pairs[t]:
                        g, is_start, is_stop, kk = pair_info[p]
                        gi = g % GC
                        Mt = mpool.tile([128, GRP], BF16, tag="M")
                        nc.vector.tensor_scalar(
                            Mt[:],
                            iota_t[:],
                            doff_t[:, p : p + 1],
                            s_t[:, t : t + 1],
                            op0=mybir.AluOpType.is_equal,
                            op1=mybir.AluOpType.mult,
                        )
                        if is_start:
                            psA = psegp.tile(
                                [128, GRP], F32, tag=f"sA{gi}", name=f"psA{gi}"
                            )
                            psB = psegp.tile(
                                [128, GRP], F32, tag=f"sB{gi}", name=f"psB{gi}"
                            )
                            cur_ps[gi] = (psA, psB)
                        psA, psB = cur_ps[gi]
                        nc.tensor.matmul(
                            psA[:], Gt[:kk, j, 0:128], Mt[:kk, :],
                            start=is_start, stop=is_stop,
                        )
                        nc.tensor.matmul(
                            psB[:], Gt[:kk, j, 128:256], Mt[:kk, :],
                            start=is_start, stop=is_stop,
                        )
                        if not is_stop:
                            continue
                        aggA = asbp.tile([128, GRP], BF16, tag="aggA")
                        aggB = asbp.tile([128, GRP], BF16, tag="aggB")
                        nc.scalar.activation(
                            aggA[:], psA[:], mybir.ActivationFunctionType.Copy
                        )
                        nc.scalar.activation(
                            aggB[:], psB[:], mybir.ActivationFunctionType.Copy
                        )
                        pso = pwp.tile([OUT, GRP], F32, tag="w")
                        nc.tensor.matmul(
                            pso[:], wts[0][:], aggA[:], start=True, stop=False
                        )
                        nc.tensor.matmul(
                            pso[:], wts[1][:], aggB[:], start=False, stop=True
                        )
                        w = GRP if g < NG - 1 else last_w
                        nc.scalar.activation(
                            agg_t[:, g, :w],
                            pso[:, :w],
                            mybir.ActivationFunctionType.Relu,
                            bias=btc_t[:, 0:1],
                            accum_out=stats_s[:, g : g + 1],
                        )
                        sqt = sqp.tile([OUT, GRP], BF16, tag="sq")
                        nc.scalar.activation(
                            sqt[:, :w],
                            agg_t[:, g, :w],
                            mybir.ActivationFunctionType.Square,
                            accum_out=stats_q[:, g : g + 1],
                        )

            # ---- BN stats AllReduce + affine finalize ----
            stsb = cpool.tile([OUT, 2], F32)
            nc.vector.tensor_reduce(
                stsb[:, 0:1], stats_s[:], mybir.AxisListType.X, mybir.AluOpType.add
            )
            nc.vector.tensor_reduce(
                stsb[:, 1:2], stats_q[:], mybir.AxisListType.X, mybir.AluOpType.add
            )
            nc.sync.dma_start(stats_in[:], stsb[:])
            nc.gpsimd.collective_compute(
                "AllGather",
                mybir.AluOpType.bypass,
                replica_groups=[list(range(C))],
                ins=[stats_in[:]],
                outs=[stats_out[:]],
            )
            stall = cpool.tile([OUT, C, 2], F32)
            nc.sync.dma_start(
                stall[:], stats_out[:].rearrange("(c p) f -> p c f", c=C)
            )
            strb = cpool.tile([OUT, 2], F32)
            nc.vector.tensor_reduce(
                strb[:],
                stall[:].rearrange("p c f -> p f c"),
                mybir.AxisListType.X,
                mybir.AluOpType.add,
            )

            mu = cpool.tile([OUT, 1], F32)
            ex2 = cpool.tile([OUT, 1], F32)
            var = cpool.tile([OUT, 1], F32)
            S_t = cpool.tile([OUT, 1], F32)
            T_t = cpool.tile([OUT, 1], F32)
            inv_n = 1.0 / float(N)
            nc.scalar.activation(
                mu[:], strb[:, 0:1], mybir.ActivationFunctionType.Copy, scale=inv_n
            )
            nc.scalar.activation(
                ex2[:], strb[:, 1:2], mybir.ActivationFunctionType.Copy, scale=inv_n
            )
            nc.scalar.activation(var[:], mu[:], mybir.ActivationFunctionType.Square)
            nc.vector.tensor_sub(var[:], ex2[:], var[:])
            # var <- rsqrt(var + eps) (ACT Rsqrt is banned for accuracy)
            nc.scalar.activation(
                var[:], var[:], mybir.ActivationFunctionType.Copy,
                bias=float(cfg["EPS"]),
            )
            nc.vector.reciprocal(var[:], var[:])
            nc.scalar.activation(var[:], var[:], mybir.ActivationFunctionType.Sqrt)
            nc.vector.tensor_mul(S_t[:], gmc_t[:], var[:])
            nc.vector.tensor_mul(T_t[:], mu[:], S_t[:])
            nc.vector.tensor_sub(T_t[:], bbc_t[:], T_t[:])

            # ---- apply affine: y = h*S + T (ACT/DVE quarters, DMA
            # interleaved so the writeback overlaps the affine) ----
            ypadT_view = ypadT_d[:].rearrange("p (g f) -> p g f", g=NG)
            qs = [0, NG // 4, NG // 2, 3 * NG // 4, NG]
            for qi in range(4):
                a, z = qs[qi], qs[qi + 1]
                if qi % 2 == 0:
                    nc.scalar.activation(
                        agg_t[:, a:z, :],
                        agg_t[:, a:z, :],
                        mybir.ActivationFunctionType.Identity,
                        bias=T_t[:, 0:1],
                        scale=S_t[:, 0:1],
                    )
                else:
                    nc.vector.tensor_scalar(
                        agg_t[:, a:z, :],
                        agg_t[:, a:z, :],
                        S_t[:, 0:1],
                        T_t[:, 0:1],
                        op0=mybir.AluOpType.mult,
                        op1=mybir.AluOpType.add,
                    )
                nc.sync.dma_start(ypadT_view[:, a:z, :], agg_t[:, a:z, :])

    nc.compile()
    return nc


def kernel(x, src, dst, W, b, gamma, beta):
    global LAST_RESULTS
    cfg = CFG
    N, E, IN, OUT, C = cfg["N"], cfg["E"], cfg["IN"], cfg["OUT"], cfg["NCORES"]
    GRP, XB, NBANKS = cfg["GRP"], cfg["XB"], cfg["NBANKS"]
    assert x.shape == (N, IN) and W.shape == (IN, OUT)
    assert src.shape == (E,) and dst.shape == (E,)

    meta, gidx_cores, doff_cores, dgo_cores, dgi_cores = _preprocess(cfg, src, dst)
    NPC, NG = meta["NPC"], meta["NG"]
    newpos = meta["newpos"]
    XK = _ceil_div(IN, 128)

    nc = _build_nc(cfg, meta)

    import ml_dtypes

    xbf = np.asarray(x, np.float32).astype(ml_dtypes.bfloat16)  # [N, IN]
    Wn = np.asarray(W, np.float32)

    iota = np.tile(
        np.arange(GRP, dtype=np.float32)[None, :], (128, 1)
    ).astype(ml_dtypes.bfloat16)
    btc = np.ascontiguousarray(np.asarray(b, np.float32)[:, None])
    gmc = np.ascontiguousarray(np.asarray(gamma, np.float32)[:, None])
    bbc = np.ascontiguousarray(np.asarray(beta, np.float32)[:, None])

    xbanks = {
        f"xb{q}": np.ascontiguousarray(xbf[q * XB : (q + 1) * XB, :])
        for q in range(NBANKS)
    }
    wmap = {
        f"wt{j}": np.ascontiguousarray(
            Wn[j * 128 : (j + 1) * 128, :]
        ).astype(ml_dtypes.bfloat16)
        for j in range(XK)
    }

    in_maps = []
    for k in range(C):
        im = {
            "gidx": gidx_cores[k],
            "doff": doff_cores[k],
            "dgo": dgo_cores[k].astype(ml_dtypes.bfloat16),
            "dgi": dgi_cores[k].astype(ml_dtypes.bfloat16),
            "iota": iota,
            "btc": btc,
            "gmc": gmc,
            "bbc": bbc,
        }
        im.update(xbanks)
        im.update(wmap)
        in_maps.append(im)

    if cfg.get("SIM"):
        from concourse.bass_interp import MultiCoreSim

        sim = MultiCoreSim(nc, num_cores=C)
        for k, core_sim in sim.cores.items():
            for name, val in in_maps[k].items():
                core_sim.tensor(name)[:] = val
        sim.simulate()
        ycomp = np.empty((N, OUT), np.float32)
        for k in range(C):
            ycomp[k * NPC : (k + 1) * NPC] = (
                sim.cores[k].tensor("ypadT")[:, :NPC].astype(np.float32).T
            )
        return ycomp[newpos]

    global LAST_NC, LAST_RUN_S
    LAST_NC = nc
    import time as _time

    _t0 = _time.time()
    res = bass_utils.run_bass_kernel_spmd(
        nc,
        in_maps,
        core_ids=list(range(C)),
        trace=cfg.get("TRACE", False),
    )
    LAST_RUN_S = _time.time() - _t0
    LAST_RESULTS = res

    ycomp = np.empty((N, OUT), np.float32)
    for k in range(C):
        ycomp[k * NPC : (k + 1) * NPC] = (
            res.results[k]["ypadT"][:, :NPC].astype(np.float32).T
        )
    return ycomp[newpos]


# revision 37
# speedup vs baseline: 3.5442x; 1.0064x over previous
"""GCN block (GraphConv + BatchNorm1d + ReLU) on 8 Trainium2 NeuronCores.

Strategy: partition dst nodes across the 8 cores; every core keeps the FULL
x table (an external input, so it is staged for free) in its HBM and gathers
x[src] rows directly — the weight is applied AFTER aggregation, which is
exact because aggregation is linear:

    agg[d] = sum_e  nsrc[src_e] * ndst[d] * x[src_e]        (segmented sum)
    y_pre[d] = agg[d] @ W + b ; h = relu(y_pre) ; BN(h)

This removes the h AllGather of the original design entirely (it cost
~700us of collective time on the critical path).

Layout is feature-major on chip ([feature, node]) so bias and the BN affine
are per-partition ACT ops, and BN batch sums fall out of the ACT
accumulator for free.

The dst->core assignment is ours to choose, so nodes are assigned to
(core, group) slots by a degree-profile-aware round-robin (nodes with equal
per-bank in-degree profiles are dealt cyclically across all 784 slots).
This equalizes every (group, bank) bucket's edge count across the 8 cores,
collapsing the shared-NEFF padding slack from ~11% to ~2%; buckets are then
padded to 16-slot granularity (the dma_gather index wrap).  128-edge blocks
may straddle group boundaries: each (block, group) pair in the shared
schedule gets its own one-hot matrix M built from a per-pair pre-shifted
dst-offset column (values outside [0,128) give zero columns, so foreign
and pad slots contribute exactly 0).  Per-edge degree norms are folded into
M by the dual-op tensor_scalar (is_equal then mult) at zero extra cost.

Groups are processed in chunks of GC=3: each group owns a dedicated
PSUM-bank pair whose accumulation chain spans all 4 src banks of its chunk
(6 seg banks + 2 W banks = all 8).  On group completion: agg pair -> SBUF
bf16, W matmul, relu(+bias ptr) with ACT accumulator emitting BN sums,
Square pass emitting sum-of-squares; AllReduce [128,2]; per-partition
affine; y^T written to HBM (host permutes rows back).

Host-side work is limited to integer index bookkeeping and layout/dtype
transforms. All floating-point math runs on device.
"""
import math
import os
import sys

sys.path.insert(0, "/opt/trn_rl_repo")

import numpy as np

import concourse.bacc as bacc
import concourse.bass as bass
import concourse.mybir as mybir
import concourse.tile as tile
from concourse import bass_utils

F32 = mybir.dt.float32
BF16 = mybir.dt.bfloat16
I16 = mybir.dt.int16

CFG = dict(
    N=100000,
    E=1600000,
    IN=256,
    OUT=128,
    NCORES=8,
    GRP=128,          # dst nodes per segment group (= one-hot free dim)
    NBANKS=4,         # src banks (bank rows must stay < 32768 for int16 idx)
    XB=25000,         # rows per x bank
    GC=3,             # groups per chunk (2*GC psum seg banks + 2 W banks <= 8)
    BATCH_BLOCKS=48,  # gather batch cap in 128-edge blocks
    EPS=1e-5,
    TRACE=False,
)

LAST_RESULTS = None  # set by kernel() for test harness introspection
LAST_NC = None
LAST_RUN_S = None


def _ceil_div(a, b):
    return (a + b - 1) // b


def _wrap16(idx, ncols):
    """int16 idx list -> [128, ncols] tile: idx i at [i%16, i//16], replicated
    8x across the 16-partition groups (one copy per GpSimd Q7 core)."""
    n = idx.shape[0]
    assert n == ncols * 16
    w = np.ascontiguousarray(idx.reshape(ncols, 16).T)
    return np.tile(w, (8, 1))


def _balance_nodes(cfg, src, dst):
    """Assign dst nodes to (core, group) slots so that every (group-pos,
    bank) bucket has a near-equal edge count on all 8 cores.  Nodes with
    identical per-bank in-degree profiles are dealt round-robin across all
    slots.  Returns newpos[node] (position in the concatenated core
    layout)."""
    N = cfg["N"]
    C, NG, GRP, XB = cfg["NCORES"], _ceil_div(N // cfg["NCORES"], cfg["GRP"]), cfg["GRP"], cfg["XB"]
    NPC = N // C
    NS = C * NG
    last_w = NPC - (NG - 1) * GRP

    bank_e = src // XB
    prof = np.bincount(dst * 4 + bank_e, minlength=N * 4).reshape(N, 4)
    _, inv = np.unique(prof, axis=0, return_inverse=True)
    order_nodes = np.argsort(inv, kind="stable")
    cls_sorted = inv[order_nodes]
    seg = np.flatnonzero(np.diff(cls_sorted)) + 1
    seg_starts = np.concatenate([[0], seg, [N]])

    slot_of = np.empty(N, np.int64)
    ptr = 0
    for i in range(len(seg_starts) - 1):
        a, b = seg_starts[i], seg_starts[i + 1]
        m = b - a
        slot_of[order_nodes[a:b]] = (np.arange(m) + ptr) % NS
        ptr = (ptr + m) % NS
    # capacity fix: slots (k, NG-1) hold only last_w nodes
    cap = np.full(NS, GRP, np.int64)
    cap[(NG - 1) * C :] = last_w  # slot id s: gp = s // C, core = s % C
    by_slot = np.argsort(slot_of, kind="stable")
    fill = np.bincount(slot_of, minlength=NS)
    cum = np.concatenate([[0], np.cumsum(fill)])
    moved = []
    for s in range(NS):
        if fill[s] > cap[s]:
            moved.extend(by_slot[cum[s] + cap[s] : cum[s + 1]])
    if moved:
        room_slots = np.repeat(
            np.arange(NS), np.maximum(cap - fill, 0)
        )[: len(moved)]
        slot_of[np.array(moved)] = room_slots
    # final positions
    by_slot = np.argsort(slot_of, kind="stable")
    fill = np.bincount(slot_of, minlength=NS)
    assert (fill == cap).all()
    offs = np.arange(N) - np.repeat(
        np.concatenate([[0], np.cumsum(fill)])[:-1], fill
    )
    s_sorted = slot_of[by_slot]
    newpos = np.empty(N, np.int64)
    newpos[by_slot] = (s_sorted % C) * NPC + (s_sorted // C) * GRP + offs
    return newpos


def _preprocess(cfg, src, dst):
    """Bucket edges by (owner core, chunk, src bank, group) under the
    balanced node assignment; build the shared (block, group) pair schedule
    and per-core gather-index / dst-offset / per-edge-degree arrays."""
    N, E = cfg["N"], cfg["E"]
    C, NBANKS, GRP, GC = cfg["NCORES"], cfg["NBANKS"], cfg["GRP"], cfg["GC"]
    XB = cfg["XB"]
    NPC = N // C
    NG = _ceil_div(NPC, GRP)
    NCH = _ceil_div(NG, GC)
    assert XB * NBANKS == N and XB < 32768

    src = src.astype(np.int64)
    dst = dst.astype(np.int64)
    deg_out = np.bincount(src, minlength=N).astype(np.float32)
    deg_in = np.bincount(dst, minlength=N).astype(np.float32)

    newpos = _balance_nodes(cfg, src, dst)
    dstN = newpos[dst]

    owner = dstN // NPC
    g_of = (dstN % NPC) // GRP
    ch_of = g_of // GC
    gi_of = g_of - ch_of * GC
    bank = src // XB
    key = ((owner * NCH + ch_of) * NBANKS + bank) * GC + gi_of
    order = np.argsort(key, kind="stable")
    s_src = src[order]
    s_dstN = dstN[order]
    s_dstO = dst[order]
    s_key = key[order]

    nkey = C * NCH * NBANKS * GC
    counts = np.bincount(key, minlength=nkey).reshape(C, NCH, NBANKS, GC)
    cmax = counts.max(axis=0)  # [NCH, NBANKS, GC]
    R = ((cmax + 15) // 16) * 16
    for ch in range(NCH):
        ngr = min(GC, NG - ch * GC)
        R[ch, 0, :ngr] = np.maximum(R[ch, 0, :ngr], 16)  # bank-0 run exists
        R[ch, :, ngr:] = 0

    # stream layout: per (chunk, bank): runs at 16-slot granularity, stream
    # rounded up to whole 128-slot blocks
    run_off = {}
    stream_blk0 = {}
    pos = 0
    for ch in range(NCH):
        ngr = min(GC, NG - ch * GC)
        for b in range(NBANKS):
            assert pos % 128 == 0
            stream_blk0[(ch, b)] = pos // 128
            for gi in range(ngr):
                if R[ch, b, gi] > 0:
                    run_off[(ch, b, gi)] = pos
                    pos += int(R[ch, b, gi])
            pos = _ceil_div(pos, 128) * 128
    nidx_tot = pos
    nb_tot = nidx_tot // 128

    # (block, group) pair schedule + per-group chain flags
    pairs = []           # (t, g)
    group_pairs = {}     # g -> [pair indices in emission order]
    block_pairs = [[] for _ in range(nb_tot)]
    for ch in range(NCH):
        ngr = min(GC, NG - ch * GC)
        for b in range(NBANKS):
            for gi in range(ngr):
                if R[ch, b, gi] == 0:
                    continue
                g = ch * GC + gi
                r0 = run_off[(ch, b, gi)]
                r1 = r0 + int(R[ch, b, gi])
                for t in range(r0 // 128, (r1 - 1) // 128 + 1):
                    p = len(pairs)
                    pairs.append((t, g))
                    group_pairs.setdefault(g, []).append(p)
                    block_pairs[t].append(p)
    npairs = len(pairs)
    # valid slot count per stream tail block (gather is trimmed to the
    # 16-granular used length; matmuls must not read unwritten Gt rows)
    used_end_of = {}
    for ch in range(NCH):
        ngr = min(GC, NG - ch * GC)
        for b in range(NBANKS):
            ue = max(
                (run_off[(ch, b, gi)] + int(R[ch, b, gi]))
                for gi in range(ngr)
                if R[ch, b, gi] > 0
            )
            used_end_of[(ch, b)] = ue
    blk_kk = np.full(nb_tot, 128, np.int64)
    for (ch, b), ue in used_end_of.items():
        t = (ue - 1) // 128
        if ue - t * 128 < 128:
            blk_kk[t] = ue - t * 128
    pair_info = []
    starts_set = {gp[0] for gp in group_pairs.values()}
    stops_set = {gp[-1] for gp in group_pairs.values()}
    for p, (t, g) in enumerate(pairs):
        pair_info.append((g, p in starts_set, p in stops_set, int(blk_kk[t])))
    # dedupe: a straddling run can emit two pairs (t, g) for consecutive
    # runs of the same g in different banks mapping to the same t — they
    # are distinct pairs (per-bank), which is fine for the psum chain.

    # gather batches: consecutive blocks within one (chunk, bank) stream.
    # nidx (16-granular) trims the stream-tail pad slots from the gather:
    # those slots have no (block, group) pairs, so they are never read.
    batches = []  # (bank, first_block, n_blocks, n_gather_idxs)
    for ch in range(NCH):
        ngr = min(GC, NG - ch * GC)
        for b in range(NBANKS):
            t0 = stream_blk0[(ch, b)]
            t1 = stream_blk0.get((ch, b + 1))
            if t1 is None:
                t1 = stream_blk0.get((ch + 1, 0), nb_tot)
            used_end = max(
                run_off[(ch, b, gi)] + int(R[ch, b, gi])
                for gi in range(ngr)
                if R[ch, b, gi] > 0
            )
            rem = t1 - t0
            t = t0
            while rem > 0:
                nb = min(rem, cfg["BATCH_BLOCKS"])
                nidx = min(nb * 128, max(used_end - t * 128, 0))
                if nidx > 0:
                    batches.append((b, t, nb, nidx))
                t += nb
                rem -= nb
    # split the final batch so the tail dependency chain drains sooner
    if batches and batches[-1][2] >= 4:
        b, t, nb, nidx = batches.pop()
        nb1 = nb // 2
        n1 = min(nb1 * 128, nidx)
        batches.append((b, t, nb1, n1))
        if nidx > n1:
            batches.append((b, t + nb1, nb - nb1, nidx - n1))

    # per (k, ch, b, gi) boundaries in the sorted edge stream
    bkeys = np.arange(nkey)
    bstarts = np.searchsorted(s_key, bkeys).reshape(C, NCH, NBANKS, GC)
    bends = np.searchsorted(s_key, bkeys, side="right").reshape(C, NCH, NBANKS, GC)

    gidx_cores = []
    doff_cores = []
    dgo_cores = []
    dgi_cores = []
    for k in range(C):
        gidx = np.zeros(nidx_tot, np.int16)
        dmod = np.full(nidx_tot, -1.0e6, np.float32)
        dgo = np.ones(nidx_tot, np.float32)
        dgi = np.ones(nidx_tot, np.float32)
        for (ch, b, gi), p0 in run_off.items():
            s, e = int(bstarts[k, ch, b, gi]), int(bends[k, ch, b, gi])
            cnt = e - s
            if cnt == 0:
                continue
            gidx[p0 : p0 + cnt] = (s_src[s:e] % XB).astype(np.int16)
            dmod[p0 : p0 + cnt] = (s_dstN[s:e] % NPC).astype(np.float32)
            dgo[p0 : p0 + cnt] = deg_out[s_src[s:e]]
            dgi[p0 : p0 + cnt] = deg_in[s_dstO[s:e]]
        dmod2 = dmod.reshape(nb_tot, 128)
        doff = np.empty((npairs, 128), np.float32)
        for p, (t, g) in enumerate(pairs):
            doff[p] = dmod2[t] - np.float32(g * GRP)
        gidx_cores.append(_wrap16(gidx, nidx_tot // 16))
        doff_cores.append(np.ascontiguousarray(doff.T))
        dgo_cores.append(np.ascontiguousarray(dgo.reshape(nb_tot, 128).T))
        dgi_cores.append(np.ascontiguousarray(dgi.reshape(nb_tot, 128).T))

    meta = dict(
        NPC=NPC,
        NG=NG,
        nidx_tot=nidx_tot,
        nb_tot=nb_tot,
        npairs=npairs,
        pairs=pairs,
        pair_info=pair_info,
        block_pairs=block_pairs,
        batches=batches,
        newpos=newpos,
    )
    return meta, gidx_cores, doff_cores, dgo_cores, dgi_cores


def _build_nc(cfg, meta):
    N, IN, OUT, C = cfg["N"], cfg["IN"], cfg["OUT"], cfg["NCORES"]
    GRP, NBANKS, XB, GC = cfg["GRP"], cfg["NBANKS"], cfg["XB"], cfg["GC"]
    NPC, NG = meta["NPC"], meta["NG"]
    nidx_tot, nb_tot = meta["nidx_tot"], meta["nb_tot"]
    npairs = meta["npairs"]
    pair_info = meta["pair_info"]
    block_pairs = meta["block_pairs"]
    batches = meta["batches"]
    XK = _ceil_div(IN, 128)
    assert OUT == 128 and GRP == 128 and IN == 256
    last_w = NPC - (NG - 1) * GRP  # valid dst cols in the last group

    nc = bacc.Bacc(
        "TRN2", target_bir_lowering=False, debug=False, num_devices=C
    )

    # ---- external inputs ----
    xb = [
        nc.dram_tensor(f"xb{q}", [XB, IN], BF16, kind="ExternalInput")
        for q in range(NBANKS)
    ]
    wt = [
        nc.dram_tensor(f"wt{j}", [128, OUT], BF16, kind="ExternalInput")
        for j in range(XK)
    ]
    gidx_d = nc.dram_tensor("gidx", [128, nidx_tot // 16], I16, kind="ExternalInput")
    doff_d = nc.dram_tensor("doff", [128, npairs], F32, kind="ExternalInput")
    dgo_d = nc.dram_tensor("dgo", [128, nb_tot], BF16, kind="ExternalInput")
    dgi_d = nc.dram_tensor("dgi", [128, nb_tot], BF16, kind="ExternalInput")
    iota_d = nc.dram_tensor("iota", [128, GRP], BF16, kind="ExternalInput")
    btc_d = nc.dram_tensor("btc", [OUT, 1], F32, kind="ExternalInput")
    gmc_d = nc.dram_tensor("gmc", [OUT, 1], F32, kind="ExternalInput")
    bbc_d = nc.dram_tensor("bbc", [OUT, 1], F32, kind="ExternalInput")

    ypadT_d = nc.dram_tensor("ypadT", [OUT, NG * GRP], BF16, kind="ExternalOutput")

    bmax = max(nb for _, _, nb, _ in batches)

    with tile.TileContext(nc) as tc:
        with (
            tc.tile_pool(name="const", bufs=1) as cpool,
            tc.tile_pool(name="dram", bufs=1, space="DRAM") as dpool,
            tc.tile_pool(name="agg", bufs=1) as apool,
            tc.tile_pool(name="gath", bufs=6) as gpool,
            tc.tile_pool(name="mpool", bufs=10) as mpool,
            tc.tile_pool(name="asb", bufs=3) as asbp,
            tc.tile_pool(name="sq", bufs=3) as sqp,
            tc.tile_pool(name="pseg", bufs=1, space="PSUM") as psegp,
            tc.tile_pool(name="pw", bufs=2, space="PSUM") as pwp,
        ):
            # ---- constants / small tiles ----
            iota_t = cpool.tile([128, GRP], BF16)
            btc_t = cpool.tile([OUT, 1], F32)
            gmc_t = cpool.tile([OUT, 1], F32)
            bbc_t = cpool.tile([OUT, 1], F32)
            gidx_t = cpool.tile([128, nidx_tot // 16], I16)
            doff_t = cpool.tile([128, npairs], F32)
            dgo_t = cpool.tile([128, nb_tot], BF16)
            dgi_t = cpool.tile([128, nb_tot], BF16)
            s_t = cpool.tile([128, nb_tot], F32)
            r2_t = cpool.tile([128, nb_tot], F32)
            stats_s = cpool.tile([OUT, NG], F32)
            stats_q = cpool.tile([OUT, NG], F32)
            wts = []
            for j in range(XK):
                wtile = cpool.tile([128, OUT], BF16, name=f"wt_s{j}")
                nc.sync.dma_start(wtile[:], wt[j][:])
                wts.append(wtile)

            # split the big index load so the first gather batches and the
            # first M-gens don't wait for the whole-tensor DMAs
            gcols = nidx_tot // 16
            gsplits = [0, min(256, gcols), min(2048, gcols), gcols]
            for a, z in zip(gsplits, gsplits[1:]):
                if z > a:
                    nc.sync.dma_start(gidx_t[:, a:z], gidx_d[:, a:z])
            nc.sync.dma_start(iota_t[:], iota_d[:])
            bsplit = [0, min(256, nb_tot), nb_tot]
            for a, z in zip(bsplit, bsplit[1:]):
                if z > a:
                    nc.sync.dma_start(dgo_t[:, a:z], dgo_d[:, a:z])
                    nc.sync.dma_start(dgi_t[:, a:z], dgi_d[:, a:z])
            psplit = [0, min(256, npairs), npairs]
            for a, z in zip(psplit, psplit[1:]):
                if z > a:
                    nc.sync.dma_start(doff_t[:, a:z], doff_d[:, a:z])
            nc.sync.dma_start(btc_t[:], btc_d[:])
            nc.sync.dma_start(gmc_t[:], gmc_d[:])
            nc.sync.dma_start(bbc_t[:], bbc_d[:])

            # per-edge norm scale s = rsqrt(max(dgo,1)) * rsqrt(max(dgi,1))
            # in two column segments so early blocks unblock fast
            for a, z in zip(bsplit, bsplit[1:]):
                if z <= a:
                    continue
                for deg_t, out_t in ((dgo_t, s_t), (dgi_t, r2_t)):
                    nc.vector.tensor_scalar(
                        out_t[:, a:z], deg_t[:, a:z], 1.0, None,
                        op0=mybir.AluOpType.max,
                    )
                    nc.vector.reciprocal(out_t[:, a:z], out_t[:, a:z])
                    nc.scalar.activation(
                        out_t[:, a:z], out_t[:, a:z],
                        mybir.ActivationFunctionType.Sqrt,
                    )
                nc.vector.tensor_mul(s_t[:, a:z], s_t[:, a:z], r2_t[:, a:z])

            # h table (feature-major, bf16): agg_t[:, g, d] = h[o, g*128+d]
            agg_t = apool.tile([OUT, NG, GRP], BF16)
            # zero the last group's pad columns (stats square-pass reads them)
            nc.gpsimd.memset(agg_t[:, NG - 1, :], 0.0)

            # internal DRAM for the BN-stats collective (AllGather is ~2x
            # cheaper than AllReduce in fixed cost; reduce locally instead)
            stats_in = dpool.tile([OUT, 2], F32)
            stats_out = dpool.tile([C * OUT, 2], F32, addr_space="Shared")

            # ---- main loop: gather + one-hot matmul segmented sum ----
            cur_ps = {}  # gi -> (psA, psB)
            for bank, t0, nblk, nidx in batches:
                Gt = gpool.tile([128, bmax, IN], BF16, tag="G")
                nc.gpsimd.dma_gather(
                    Gt[:, : _ceil_div(nidx, 128), :],
                    xb[bank][:],
                    gidx_t[:, t0 * 8 : t0 * 8 + nidx // 16],
                    nidx,
                    nidx,
                    IN,
                    single_packet=False,
                )
                for j in range(nblk):
                    t = t0 + j
                    for p in block_pairs[t]:
                        g, is_start, is_stop, kk = pair_info[p]
                        gi = g % GC
                        Mt = mpool.tile([128, GRP], BF16, tag="M")
                        nc.vector.tensor_scalar(
                            Mt[:],
                            iota_t[:],
                            doff_t[:, p : p + 1],
                            s_t[:, t : t + 1],
                            op0=mybir.AluOpType.is_equal,
                            op1=mybir.AluOpType.mult,
                        )
                        if is_start:
                            psA = psegp.tile(
                                [128, GRP], F32, tag=f"sA{gi}", name=f"psA{gi}"
                            )
                            psB = psegp.tile(
                                [128, GRP], F32, tag=f"sB{gi}", name=f"psB{gi}"
                            )
                            cur_ps[gi] = (psA, psB)
                        psA, psB = cur_ps[gi]
                        nc.tensor.matmul(
                            psA[:], Gt[:kk, j, 0:128], Mt[:kk, :],
                            start=is_start, stop=is_stop,
                        )
                        nc.tensor.matmul(
                            psB[:], Gt[:kk, j, 128:256], Mt[:kk, :],
                            start=is_start, stop=is_stop,
                        )
                        if not is_stop:
                            continue
                        aggA = asbp.tile([128, GRP], BF16, tag="aggA")
                        aggB = asbp.tile([128, GRP], BF16, tag="aggB")
                        nc.scalar.activation(
                            aggA[:], psA[:], mybir.ActivationFunctionType.Copy
                        )
                        nc.scalar.activation(
                            aggB[:], psB[:], mybir.ActivationFunctionType.Copy
                        )
                        pso = pwp.tile([OUT, GRP], F32, tag="w")
                        nc.tensor.matmul(
                            pso[:], wts[0][:], aggA[:], start=True, stop=False
                        )
                        nc.tensor.matmul(
                            pso[:], wts[1][:], aggB[:], start=False, stop=True
                        )
                        w = GRP if g < NG - 1 else last_w
                        nc.scalar.activation(
                            agg_t[:, g, :w],
                            pso[:, :w],
                            mybir.ActivationFunctionType.Relu,
                            bias=btc_t[:, 0:1],
                            accum_out=stats_s[:, g : g + 1],
                        )
                        sqt = sqp.tile([OUT, GRP], BF16, tag="sq")
                        nc.scalar.activation(
                            sqt[:, :w],
                            agg_t[:, g, :w],
                            mybir.ActivationFunctionType.Square,
                            accum_out=stats_q[:, g : g + 1],
                        )

            # ---- BN stats AllReduce + affine finalize ----
            stsb = cpool.tile([OUT, 2], F32)
            nc.vector.tensor_reduce(
                stsb[:, 0:1], stats_s[:], mybir.AxisListType.X, mybir.AluOpType.add
            )
            nc.vector.tensor_reduce(
                stsb[:, 1:2], stats_q[:], mybir.AxisListType.X, mybir.AluOpType.add
            )
            nc.sync.dma_start(stats_in[:], stsb[:])
            nc.gpsimd.collective_compute(
                "AllGather",
                mybir.AluOpType.bypass,
                replica_groups=[list(range(C))],
                ins=[stats_in[:]],
                outs=[stats_out[:]],
            )
            stall = cpool.tile([OUT, C, 2], F32)
            nc.sync.dma_start(
                stall[:], stats_out[:].rearrange("(c p) f -> p c f", c=C)
            )
            strb = cpool.tile([OUT, 2], F32)
            nc.vector.tensor_reduce(
                strb[:],
                stall[:].rearrange("p c f -> p f c"),
                mybir.AxisListType.X,
                mybir.AluOpType.add,
            )

            mu = cpool.tile([OUT, 1], F32)
            ex2 = cpool.tile([OUT, 1], F32)
            var = cpool.tile([OUT, 1], F32)
            S_t = cpool.tile([OUT, 1], F32)
            T_t = cpool.tile([OUT, 1], F32)
            inv_n = 1.0 / float(N)
            nc.scalar.activation(
                mu[:], strb[:, 0:1], mybir.ActivationFunctionType.Copy, scale=inv_n
            )
            nc.scalar.activation(
                ex2[:], strb[:, 1:2], mybir.ActivationFunctionType.Copy, scale=inv_n
            )
            nc.scalar.activation(var[:], mu[:], mybir.ActivationFunctionType.Square)
            nc.vector.tensor_sub(var[:], ex2[:], var[:])
            # var <- rsqrt(var + eps) (ACT Rsqrt is banned for accuracy)
            nc.scalar.activation(
                var[:], var[:], mybir.ActivationFunctionType.Copy,
                bias=float(cfg["EPS"]),
            )
            nc.vector.reciprocal(var[:], var[:])
            nc.scalar.activation(var[:], var[:], mybir.ActivationFunctionType.Sqrt)
            nc.vector.tensor_mul(S_t[:], gmc_t[:], var[:])
            nc.vector.tensor_mul(T_t[:], mu[:], S_t[:])
            nc.vector.tensor_sub(T_t[:], bbc_t[:], T_t[:])

            # ---- apply affine: y = h*S + T (ACT/DVE quarters, DMA
            # interleaved so the writeback overlaps the affine) ----
            ypadT_view = ypadT_d[:].rearrange("p (g f) -> p g f", g=NG)
            qs = [0, NG // 4, NG // 2, 3 * NG // 4, NG]
            for qi in range(4):
                a, z = qs[qi], qs[qi + 1]
                if qi % 2 == 0:
                    nc.scalar.activation(
                        agg_t[:, a:z, :],
                        agg_t[:, a:z, :],
                        mybir.ActivationFunctionType.Identity,
                        bias=T_t[:, 0:1],
                        scale=S_t[:, 0:1],
                    )
                else:
                    nc.vector.tensor_scalar(
                        agg_t[:, a:z, :],
                        agg_t[:, a:z, :],
                        S_t[:, 0:1],
                        T_t[:, 0:1],
                        op0=mybir.AluOpType.mult,
                        op1=mybir.AluOpType.add,
                    )
                nc.sync.dma_start(ypadT_view[:, a:z, :], agg_t[:, a:z, :])

    nc.compile()
    return nc


def kernel(x, src, dst, W, b, gamma, beta):
    global LAST_RESULTS
    cfg = CFG
    N, E, IN, OUT, C = cfg["N"], cfg["E"], cfg["IN"], cfg["OUT"], cfg["NCORES"]
    GRP, XB, NBANKS = cfg["GRP"], cfg["XB"], cfg["NBANKS"]
    assert x.shape == (N, IN) and W.shape == (IN, OUT)
    assert src.shape == (E,) and dst.shape == (E,)

    meta, gidx_cores, doff_cores, dgo_cores, dgi_cores = _preprocess(cfg, src, dst)
    NPC, NG = meta["NPC"], meta["NG"]
    newpos = meta["newpos"]
    XK = _ceil_div(IN, 128)

    nc = _build_nc(cfg, meta)

    import ml_dtypes

    xbf = np.asarray(x, np.float32).astype(ml_dtypes.bfloat16)  # [N, IN]
    Wn = np.asarray(W, np.float32)

    iota = np.tile(
        np.arange(GRP, dtype=np.float32)[None, :], (128, 1)
    ).astype(ml_dtypes.bfloat16)
    btc = np.ascontiguousarray(np.asarray(b, np.float32)[:, None])
    gmc = np.ascontiguousarray(np.asarray(gamma, np.float32)[:, None])
    bbc = np.ascontiguousarray(np.asarray(beta, np.float32)[:, None])

    xbanks = {
        f"xb{q}": np.ascontiguousarray(xbf[q * XB : (q + 1) * XB, :])
        for q in range(NBANKS)
    }
    wmap = {
        f"wt{j}": np.ascontiguousarray(
            Wn[j * 128 : (j + 1) * 128, :]
        ).astype(ml_dtypes.bfloat16)
        for j in range(XK)
    }

    in_maps = []
    for k in range(C):
        im = {
            "gidx": gidx_cores[k],
            "doff": doff_cores[k],
            "dgo": dgo_cores[k].astype(ml_dtypes.bfloat16),
            "dgi": dgi_cores[k].astype(ml_dtypes.bfloat16),
            "iota": iota,
            "btc": btc,
            "gmc": gmc,
            "bbc": bbc,
        }
        im.update(xbanks)
        im.update(wmap)
        in_maps.append(im)

    if cfg.get("SIM"):
        from concourse.bass_interp import MultiCoreSim

        sim = MultiCoreSim(nc, num_cores=C)
        for k, core_sim in sim.cores.items():
            for name, val in in_maps[k].items():
                core_sim.tensor(name)[:] = val
        sim.simulate()
        ycomp = np.empty((N, OUT), np.float32)
        for k in range(C):
            ycomp[k * NPC : (k + 1) * NPC] = (
                sim.cores[k].tensor("ypadT")[:, :NPC].astype(np.float32).T
            )
        return ycomp[newpos]

    global LAST_NC, LAST_RUN_S
    LAST_NC = nc
    import time as _time

    _t0 = _time.time()
    res = bass_utils.run_bass_kernel_spmd(
        nc,
        in_maps,
        core_ids=list(range(C)),
        trace=cfg.get("TRACE", False),
    )
    LAST_RUN_S = _time.time() - _t0
    LAST_RESULTS = res

    ycomp = np.empty((N, OUT), np.float32)
    for k in range(C):
        ycomp[k * NPC : (k + 1) * NPC] = (
            res.results[k]["ypadT"][:, :NPC].astype(np.float32).T
        )
    return ycomp[newpos]
